# revision 1
# baseline (speedup 1.0000x reference)
"""Self-contained Trainium2 Bass kernel for a 2-layer GCN (GraphConv + BN + ReLU + GraphConv).

v3 strategy (8 NeuronCores, SPMD) — rebuilt around measured dma_gather behavior:
  - dma_gather is Q7 descriptor-gen bound: ~2.4 ns/row clean with 4 SWDGE queues.
    Concurrent-engine SBUF traffic slows it; narrow (128-col) one-hot masks with ONE
    matmul per tile run at full gather speed, wide masks / multi-matmul tiles do not.
  - Nodes are dst-sharded per core, then PERMUTED within each core so that every
    (chunk of 128 dst nodes, src-quartile) group has <= 512 edges (4 gather tiles).
    All chunk/slot mappings are host-side data, so the permutation is free; the
    host inverts it when assembling the output.
  - Per (octet of 8 chunks, quartile): one dma_gather (queue=quartile) fetches
    hw[src] rows (256B f16). Per 128-edge tile: one DVE one-hot mask
    M[e,d] = (iota[d]==dstloc[e]) * s_in[e], one PE matmul psum[feat, dst128] += X^T M.
  - BN stats via DVE bn_stats per 4-chunk block (no ACT during gather phases:
    alternating activation functions force table reloads that stall the DMA).
  - AllReduce [128,2] for global stats; BN+ReLU+W2 per chunk; AllGather the
    64-col hw2 table; same gather machinery for layer 2 (lhsT sliced to 64 feats).
"""
import math
import os

import numpy as np

import concourse.bacc as bacc
import concourse.mybir as mybir
import concourse.tile as tile
from concourse import bass_utils

# Problem constants (hardcoded per the task contract).
N_NODES = 100000
N_EDGES = 1600000
IN_DIM = 128
HID_DIM = 128
OUT_DIM = 64
BN_EPS = 1e-5
NCORES = 8
P = 128
C = 98                  # chunks per core
BP = C * P              # padded nodes per core (12544)
B = N_NODES // NCORES   # real nodes per core (12500)
QR = 2 * BP             # table rows per quartile (25088), int16-indexable
OCT = 8                 # chunks per gather octet
NOCT = math.ceil(C / OCT)   # 13


class Plan:
    pass


def _pack_core(dv, cap):
    """Greedy assignment of nodes (rows of dv [B,4]) to C chunks of <=128
    nodes, respecting per-(chunk, quartile) edge capacities cap [C,4]."""
    tot = dv.sum(1)
    order = np.argsort(-tot)
    loads = np.zeros((C, 4), np.int64)
    counts = np.zeros(C, np.int64)
    assign = np.zeros(len(dv), np.int64)
    for i in order:
        v = dv[i]
        cand = np.nonzero(counts < P)[0]
        newl = loads[cand] + v
        over = np.maximum(newl - cap[cand], 0).sum(1)
        score = over * 100000 + newl.max(1)
        c = cand[np.argmin(score)]
        assign[i] = c
        loads[c] += v
        counts[c] += 1
    return assign, loads, counts


def _swap_repair(dv, assign, loads, cap, max_passes=10):
    """Swap nodes between chunks to push loads under the shared capacities."""
    members = [list(np.nonzero(assign == c)[0]) for c in range(C)]
    for _ in range(max_passes):
        ov = loads - cap
        over = np.argwhere(ov > 0)
        if len(over) == 0:
            break
        over = over[np.argsort(-ov[over[:, 0], over[:, 1]])]
        fixed = 0
        for c, q in over:
            guard = 0
            while loads[c, q] > cap[c, q] and guard < 8:
                guard += 1
                mem = np.array(members[c])
                mem = mem[dv[mem, q] > 0]
                if len(mem) == 0:
                    break
                i = mem[np.argmax(dv[mem, q])]
                vi = dv[i]
                best = None
                room = cap[:, q] - loads[:, q]
                for c2 in np.argsort(-room)[:24]:
                    if c2 == c or room[c2] <= 0:
                        continue
                    for j in members[c2][:48]:
                        vj = dv[j]
                        if vj[q] >= vi[q]:
                            continue
                        nc2 = loads[c2] + vi - vj
                        ncc = loads[c] - vi + vj
                        if (nc2 <= cap[c2]).all() and (ncc <= loads[c]).all():
                            best = (c2, j, nc2, ncc)
                            break
                    if best:
                        break
                if best is None:
                    break
                c2, j, nc2, ncc = best
                members[c].remove(i)
                members[c2].remove(j)
                members[c].append(j)
                members[c2].append(i)
                assign[i], assign[j] = c2, c
                loads[c2], loads[c] = nc2, ncc
                fixed += 1
        if fixed == 0:
            break
    return assign, loads


def _plan(src, dst, h_s_out, s_in_full):
    pl = Plan()
    core = dst // B
    src_core = src // B

    # per-node quartile in-degree vectors (src quartile = src core pair)
    quart_of_src = src_core // 2
    deg = np.zeros((N_NODES, 4), np.int64)
    np.add.at(deg, (dst, quart_of_src), 1)

    # shared target schedule: base 4 tiles (512 edges) per (chunk, quartile);
    # structural per-quartile excess gets extra tiles on the first chunk ranks,
    # identically on every core so the shared schedule stays tight.
    eq = np.zeros((NCORES, 4), np.int64)
    np.add.at(eq, (core, quart_of_src), 1)
    cap = np.full((C, 4), 512, np.int64)
    for q in range(4):
        excess = int(eq[:, q].max()) - C * 512
        extra_tiles = max(0, -(-excess // P)) + 1  # +1 margin
        for j in range(extra_tiles):
            cap[j % C, q] += P * (1 + j // C)

    # per-core balanced chunk assignment + slot permutation
    # slot_of[node] = position in its core's padded 12544-slot table slice
    slot_of = np.zeros(N_NODES, np.int64)
    perm = np.full((NCORES, BP), -1, np.int64)  # slot -> node id (-1 = pad)
    for r in range(NCORES):
        nodes = np.arange(r * B, (r + 1) * B)
        assign, loads, counts = _pack_core(deg[nodes], cap)
        assign, loads = _swap_repair(deg[nodes], assign, loads, cap)
        # place nodes within chunks (chunk ids are shared schedule ranks)
        off = np.zeros(C, np.int64)
        for i, cc in enumerate(assign):
            slot_of[nodes[i]] = cc * P + off[cc]
            perm[r, cc * P + off[cc]] = nodes[i]
            off[cc] += 1
    pl.perm = perm

    table_row = core * 0  # placeholder
    tr = np.zeros(N_NODES, np.int64)
    tr[:] = (np.arange(N_NODES) // B) * BP + slot_of
    pl.table_row = tr

    # edge schedule: group = (dst chunk, src quartile)
    e_core = core
    e_chunk = slot_of[dst] // P
    e_dcol = slot_of[dst] % P
    e_q = tr[src] // QR
    cnt = np.zeros((NCORES, C, 4), np.int64)
    np.add.at(cnt, (e_core, e_chunk, e_q), 1)
    T = np.ceil(cnt / P).astype(np.int64).max(axis=0)  # shared [C, 4]
    pl.T = T
    T_total = int(T.sum())
    S = T_total * P
    pl.T_total, pl.S = T_total, S

    # slot offsets: octet-major, quartile, chunk, tile
    goff = np.zeros((C, 4), np.int64)
    region = {}
    tcol = np.zeros((C, 4), np.int64)  # global tile index of (c,q) tile 0
    acc = 0
    tacc = 0
    for o in range(NOCT):
        c0, c1 = o * OCT, min((o + 1) * OCT, C)
        for q in range(4):
            r0 = acc
            for c in range(c0, c1):
                goff[c, q] = acc
                tcol[c, q] = tacc
                acc += T[c, q] * P
                tacc += T[c, q]
            region[(o, q)] = (r0, (acc - r0) // P)
    pl.goff, pl.region, pl.tcol = goff, region, tcol
    assert acc == S

    # per-core slot arrays
    srcloc = np.zeros((NCORES, S), np.int16)
    dstloc = np.full((NCORES, S), 999.0, np.float32)
    sinv = np.zeros((NCORES, S), np.float32)

    order = np.lexsort((tr[src], e_q, e_chunk, e_core))
    so, co = src[order], e_core[order]
    cho, qo = e_chunk[order], e_q[order]
    dco = e_dcol[order]
    sio = s_in_full[dst[order]]
    run_sizes = cnt.reshape(-1)
    run_starts = np.concatenate([[0], np.cumsum(run_sizes)])[:-1]
    run_id = (co * C + cho) * 4 + qo
    within = np.arange(len(order)) - run_starts[run_id]
    slots = goff[cho, qo] + within
    srcloc[co, slots] = (tr[so] - qo * QR).astype(np.int16)
    dstloc[co, slots] = dco.astype(np.float32)
    sinv[co, slots] = sio

    # wrapped idx for dma_gather: slot i -> [i % 16, i // 16], replicated x8
    w = srcloc.reshape(NCORES, S // 16, 16)
    pl.idx16 = np.ascontiguousarray(np.tile(w.transpose(0, 2, 1), (1, 8, 1)))
    # [128, T_total] layouts: slot i -> [i % 128, i // 128]
    pl.dstloc_t = np.ascontiguousarray(
        dstloc.reshape(NCORES, T_total, P).transpose(0, 2, 1))
    pl.sinv_t = np.ascontiguousarray(
        sinv.reshape(NCORES, T_total, P).transpose(0, 2, 1))

    # s_out per (core, chunk-col): [NCORES, 128, C]; pad slots -> 0
    sot = np.zeros((NCORES, BP), np.float32)
    for r in range(NCORES):
        valid = perm[r] >= 0
        sot[r, valid] = h_s_out[perm[r][valid]]
    pl.sout_t = np.ascontiguousarray(sot.reshape(NCORES, C, P).transpose(0, 2, 1))
    return pl


def _build(pl):
    f16, f32 = mybir.dt.float16, mybir.dt.float32
    i16 = mybir.dt.int16
    T, T_total, S = pl.T, pl.T_total, pl.S
    rg = [list(range(NCORES))]
    NTAB = NCORES * BP

    nc = bacc.Bacc("TRN2", target_bir_lowering=False, debug=False,
                   num_devices=NCORES, num_swdge_queues=4)

    h_d = nc.dram_tensor("h", [IN_DIM, BP], f16, kind="ExternalInput")
    w1_d = nc.dram_tensor("w1", [IN_DIM, HID_DIM], f32, kind="ExternalInput")
    w2_d = nc.dram_tensor("w2", [HID_DIM, OUT_DIM], f32, kind="ExternalInput")
    gmb_d = nc.dram_tensor("gmb", [HID_DIM, 2], f32, kind="ExternalInput")
    b2r_d = nc.dram_tensor("b2r", [P, OUT_DIM], f32, kind="ExternalInput")
    sout_d = nc.dram_tensor("sout", [P, C], f32, kind="ExternalInput")
    idx_d = nc.dram_tensor("idx", [P, S // 16], i16, kind="ExternalInput")
    dstloc_d = nc.dram_tensor("dstloc", [P, T_total], f32, kind="ExternalInput")
    sinv_d = nc.dram_tensor("sinv", [P, T_total], f32, kind="ExternalInput")
    iota_d = nc.dram_tensor("iotaf", [P, P], f16, kind="ExternalInput")
    ident_d = nc.dram_tensor("identf", [P, P], f32, kind="ExternalInput")
    out_d = nc.dram_tensor("out", [BP, OUT_DIM], f32, kind="ExternalOutput")

    hw_slice = nc.dram_tensor("hw_slice", [BP, HID_DIM], f16)
    hw_full = nc.dram_tensor("hw_full", [NTAB, HID_DIM], f16, addr_space="Shared")
    hw2_slice = nc.dram_tensor("hw2_slice", [BP, OUT_DIM], f16)
    hw2s_full = nc.dram_tensor("hw2s_full", [NTAB, OUT_DIM], f16, addr_space="Shared")
    hw2_full = nc.dram_tensor("hw2_full", [NTAB, P], f16)
    stat_in = nc.dram_tensor("stat_in", [P, 2], f32)
    stat_out = nc.dram_tensor("stat_out", [P, 2], f32, addr_space="Shared")

    AF = mybir.ActivationFunctionType
    OP = mybir.AluOpType

    with tile.TileContext(nc) as tc:
        with (
            tc.tile_pool(name="const", bufs=1) as cp,
            tc.tile_pool(name="hct", bufs=3) as htp,
            tc.tile_pool(name="evac", bufs=4) as ep,
            tc.tile_pool(name="xg", bufs=3) as xp,
            tc.tile_pool(name="mp", bufs=6) as mp,
            tc.tile_pool(name="ps_tr", bufs=2, space="PSUM") as pp_tr,
            tc.tile_pool(name="ps_agg", bufs=3, space="PSUM") as pp_agg,
            tc.tile_pool(name="ps_w2", bufs=3, space="PSUM") as pp_w2,
        ):
            # ---- constants ----
            iota_f = cp.tile([P, P], f16)
            nc.sync.dma_start(iota_f[:], iota_d[:, :])
            ident = cp.tile([P, P], f32)
            nc.sync.dma_start(ident[:], ident_d[:, :])
            w1f32 = cp.tile([IN_DIM, HID_DIM], f32)
            nc.sync.dma_start(w1f32[:], w1_d[:, :])
            w1f = cp.tile([IN_DIM, HID_DIM], f16)
            nc.vector.tensor_copy(w1f[:], w1f32[:])
            w2f32 = cp.tile([HID_DIM, OUT_DIM], f32)
            nc.sync.dma_start(w2f32[:], w2_d[:, :])
            w2f = cp.tile([HID_DIM, OUT_DIM], f16)
            nc.vector.tensor_copy(w2f[:], w2f32[:])
            gmb = cp.tile([HID_DIM, 2], f32)
            nc.sync.dma_start(gmb[:], gmb_d[:, :])
            b2r = cp.tile([P, OUT_DIM], f32)
            nc.sync.dma_start(b2r[:], b2r_d[:, :])
            sout_t = cp.tile([P, C], f32)
            nc.sync.dma_start(sout_t[:], sout_d[:, :])
            idx_t = cp.tile([P, S // 16], i16)
            nc.sync.dma_start(idx_t[:], idx_d[:, :])
            dl_t = cp.tile([P, T_total], f32)
            nc.sync.dma_start(dl_t[:], dstloc_d[:, :])
            si_t = cp.tile([P, T_total], f32)
            nc.sync.dma_start(si_t[:], sinv_d[:, :])
            H1 = cp.tile([P, BP], f16)
            S6 = cp.tile([P, 49 * 6], f32)

            # ---- stage A: hw = s_out * (h @ W1), per chunk ----
            def stage_a():
                for c in range(C):
                    hct = htp.tile([IN_DIM, P], f16, tag="hct")
                    nc.sync.dma_start(hct[:], h_d[:, c * P:(c + 1) * P])
                    psA = pp_agg.tile([P, HID_DIM], f32, tag="agg")
                    nc.tensor.matmul(psA[:], lhsT=hct[:], rhs=w1f[:], start=True, stop=True)
                    hwc = ep.tile([P, HID_DIM], f16, tag="hwc")
                    nc.vector.tensor_scalar(
                        out=hwc[:], in0=psA[:], scalar1=sout_t[:, c:c + 1], scalar2=None,
                        op0=OP.mult,
                    )
                    nc.sync.dma_start(hw_slice[c * P:(c + 1) * P, :], hwc[:, :])

            _iters = int(os.environ.get("KERNEL_TIME_ITERS", "1"))
            _wrap = os.environ.get("KERNEL_TIME_WRAP", "phase")

            def allgather1():
                nc.gpsimd.collective_compute(
                    "AllGather", OP.bypass, replica_groups=rg,
                    ins=[hw_slice.ap().opt()], outs=[hw_full.ap().opt()],
                )
                tc.strict_bb_all_engine_barrier()

            # ---- shared agg machinery: one narrow mask + one matmul per tile ----
            _clevel = int(os.environ.get("KERNEL_COMPUTE_LEVEL", "3"))
            if os.environ.get("KERNEL_SKIP_COMPUTE", "0") == "1":
                _clevel = 0
            _skip_compute = _clevel < 3
            _xbufs = int(os.environ.get("KERNEL_XBUFS", "2"))
            _mbufs = int(os.environ.get("KERNEL_MBUFS", "56"))
            _psbufs = int(os.environ.get("KERNEL_PSBUFS", "3"))

            def agg_phase(table, lhs_w, psum_pool, psum_tag, swap, epilogue):
                # Masks are built LOOKAHEAD chunks early so the DVE stays ahead
                # of the PE; epilogue evacs run on ACT, keeping DVE mask builds
                # off the psum-drain critical path.
                LOOKAHEAD = 2
                pending = {}

                def build_masks(c):
                    tiles = []
                    for q in range(4):
                        for t in range(int(T[c, q])):
                            gt = int(pl.tcol[c, q]) + t
                            Mt = mp.tile([P, P], f16, tag="M", bufs=_mbufs)
                            nc.vector.tensor_scalar(
                                out=Mt[:], in0=iota_f[:],
                                scalar1=dl_t[:, gt:gt + 1],
                                scalar2=si_t[:, gt:gt + 1],
                                op0=OP.is_equal, op1=OP.mult,
                            )
                            tiles.append((q, t, Mt))
                    pending[c] = tiles

                for o in range(NOCT):
                    c0, c1 = o * OCT, min((o + 1) * OCT, C)
                    Xq = []
                    for q in range(4):
                        r0, ntiles = pl.region[(o, q)]
                        if ntiles == 0:
                            Xq.append(None)
                            continue
                        X = xp.tile([P, ntiles, HID_DIM], f16, tag=f"Xq{q}", bufs=_xbufs)
                        nc.gpsimd.dma_gather(
                            out_ap=X[:],
                            in_ap=table[q * QR:(q + 1) * QR, :],
                            idxs_ap=idx_t[:, r0 // 16:r0 // 16 + ntiles * 8],
                            num_idxs=ntiles * P,
                            num_idxs_reg=ntiles * P,
                            elem_size=HID_DIM,
                            single_packet=False,
                            queue_num=q,
                        )
                        Xq.append(X)
                    if _clevel == 0:
                        continue
                    for c in range(c0, c1):
                        if c == 0:
                            for cc in range(min(LOOKAHEAD + 1, C)):
                                build_masks(cc)
                        elif c + LOOKAHEAD < C:
                            build_masks(c + LOOKAHEAD)
                        if _clevel == 1:
                            pending.pop(c)
                            continue
                        Tc = int(T[c].sum())
                        if swap:
                            ps = psum_pool.tile([P, lhs_w], f32, tag=psum_tag,
                                                bufs=_psbufs)
                        else:
                            ps = psum_pool.tile([lhs_w, P], f32, tag=psum_tag,
                                                bufs=_psbufs)
                        ti = 0
                        for q, t, Mt in pending.pop(c):
                            r0, _nt = pl.region[(o, q)]
                            xt0 = int((pl.goff[c, q] - r0) // P)
                            xs = Xq[q][:, xt0 + t, 0:lhs_w]
                            if swap:
                                nc.tensor.matmul(
                                    ps[:], lhsT=Mt[:], rhs=xs,
                                    start=(ti == 0), stop=(ti == Tc - 1),
                                )
                            else:
                                nc.tensor.matmul(
                                    ps[:], lhsT=xs, rhs=Mt[:],
                                    start=(ti == 0), stop=(ti == Tc - 1),
                                )
                            ti += 1
                        if _clevel >= 3:
                            epilogue(c, ps)

            # ---- phase 1: layer-1 aggregation -> H1 + bn stats ----
            def epi1(c, ps):
                # evac on ACT (constant Copy func -> no activation-table reload)
                nc.scalar.activation(out=H1[:, c * P:(c + 1) * P], in_=ps[:],
                                     func=AF.Copy)
                if c % 2 == 1:
                    # equal-count (256-col) records keep bn_aggr's variance
                    # combination exact; C=98 is even so every chunk is covered
                    blk0 = (c // 2) * 2
                    nc.vector.bn_stats(
                        S6[:, (c // 2) * 6:(c // 2) * 6 + 6],
                        H1[:, blk0 * P:(c + 1) * P],
                    )

            # ---- BN stats -> (S1, S2) -> AllReduce -> A, B ----
            ag = cp.tile([P, 2], f32)
            s12 = cp.tile([P, 2], f32)
            msq = cp.tile([P, 1], f32)
            st = cp.tile([P, 2], f32)
            mean = cp.tile([P, 1], f32)
            var = cp.tile([P, 1], f32)
            msq2 = cp.tile([P, 1], f32)
            sd = cp.tile([P, 1], f32)
            inv = cp.tile([P, 1], f32)
            A = cp.tile([P, 1], f32)
            Bb = cp.tile([P, 1], f32)

            def stats_chain():
                nc.vector.bn_aggr(ag[:], S6[:])
                # S1 = mean * BP ; S2 = (var + mean^2) * BP (pad cols are zeros)
                nc.vector.tensor_scalar(out=s12[:, 0:1], in0=ag[:, 0:1],
                                        scalar1=float(BP), scalar2=None, op0=OP.mult)
                nc.vector.tensor_tensor(out=msq[:], in0=ag[:, 0:1], in1=ag[:, 0:1],
                                        op=OP.mult)
                nc.vector.tensor_tensor(out=msq[:], in0=ag[:, 1:2], in1=msq[:], op=OP.add)
                nc.vector.tensor_scalar(out=s12[:, 1:2], in0=msq[:],
                                        scalar1=float(BP), scalar2=None, op0=OP.mult)
                nc.sync.dma_start(stat_in[:, :], s12[:])
                nc.gpsimd.collective_compute(
                    "AllReduce", OP.add, replica_groups=rg,
                    ins=[stat_in.ap().opt()], outs=[stat_out.ap().opt()],
                )
                tc.strict_bb_all_engine_barrier()
                nc.sync.dma_start(st[:], stat_out[:, :])
                nc.vector.tensor_scalar(out=mean[:], in0=st[:, 0:1], scalar1=1.0 / N_NODES,
                                        scalar2=None, op0=OP.mult)
                nc.vector.tensor_scalar(out=var[:], in0=st[:, 1:2], scalar1=1.0 / N_NODES,
                                        scalar2=None, op0=OP.mult)
                nc.vector.tensor_tensor(out=msq2[:], in0=mean[:], in1=mean[:], op=OP.mult)
                nc.vector.tensor_tensor(out=var[:], in0=var[:], in1=msq2[:], op=OP.subtract)
                nc.vector.tensor_scalar(out=sd[:], in0=var[:], scalar1=BN_EPS, scalar2=None,
                                        op0=OP.add)
                nc.scalar.activation(out=sd[:], in_=sd[:], func=AF.Sqrt)
                nc.vector.reciprocal(out=inv[:], in_=sd[:])
                nc.vector.tensor_tensor(out=A[:], in0=inv[:], in1=gmb[:, 0:1], op=OP.mult)
                nc.vector.tensor_tensor(out=Bb[:], in0=mean[:], in1=A[:], op=OP.mult)
                nc.vector.tensor_tensor(out=Bb[:], in0=gmb[:, 1:2], in1=Bb[:], op=OP.subtract)

            # ---- phase 2: BN+relu, hw2 = s_out * (t @ W2) ----
            def phase2():
                for c in range(C):
                    tcn = ep.tile([P, P], f16, tag="tcn")
                    nc.scalar.activation(out=tcn[:], in_=H1[:, c * P:(c + 1) * P],
                                         func=AF.Relu, bias=Bb[:], scale=A[:])
                    ps2 = pp_w2.tile([OUT_DIM, P], f32, tag="w2")
                    nc.tensor.matmul(ps2[:], lhsT=w2f[:], rhs=tcn[:], start=True, stop=True)
                    u = ep.tile([OUT_DIM, P], f32, tag="u")
                    nc.vector.tensor_copy(u[:], ps2[:])
                    pst2 = pp_tr.tile([P, OUT_DIM], f32, tag="ptr")
                    nc.tensor.transpose(out=pst2[:], in_=u[:], identity=ident[:OUT_DIM, :OUT_DIM])
                    hw2c = ep.tile([P, OUT_DIM], f16, tag="hw2c")
                    nc.vector.tensor_scalar(out=hw2c[:], in0=pst2[:],
                                            scalar1=sout_t[:, c:c + 1], scalar2=None,
                                            op0=OP.mult)
                    nc.sync.dma_start(hw2_slice[c * P:(c + 1) * P, :], hw2c[:, :])

            def allgather2():
                # gather the compact 64-col table (half the bytes), then one
                # local DRAM->DRAM DMA expands to the 256B-row gather layout
                nc.gpsimd.collective_compute(
                    "AllGather", OP.bypass, replica_groups=rg,
                    ins=[hw2_slice.ap().opt()], outs=[hw2s_full.ap().opt()],
                )
                tc.strict_bb_all_engine_barrier()
                h2 = NTAB // 2
                nc.sync.dma_start(hw2_full[0:h2, 0:OUT_DIM], hw2s_full[0:h2, :])
                nc.sync.dma_start(hw2_full[h2:NTAB, 0:OUT_DIM], hw2s_full[h2:NTAB, :])
                tc.strict_bb_all_engine_barrier()

            # ---- phase 3: layer-2 aggregation + b2 -> out ----
            # swapped matmul orientation: psum arrives [dst128, feat64]
            def epi3(c, ps):
                oc = ep.tile([P, OUT_DIM], f32, tag="oc")
                nc.vector.tensor_tensor(out=oc[:], in0=ps[:], in1=b2r[:], op=OP.add)
                nc.sync.dma_start(out_d[c * P:(c + 1) * P, :], oc[:, :])

            def whole():
                stage_a()
                allgather1()
                agg_phase(hw_full, HID_DIM, pp_agg, "agg", False, epi1)
                stats_chain()
                phase2()
                allgather2()
                agg_phase(hw2_full, OUT_DIM, pp_w2, "w2", True, epi3)

            _phases = set(os.environ.get("KERNEL_TIME_PHASES", "a,1,2,3").split(","))

            def rep(section, fn):
                if _iters > 1 and section in _phases:
                    with tc.For_i(0, _iters, 1):
                        fn()
                else:
                    fn()

            if _skip_compute:
                nc.vector.memset(H1[:], 0.0)
                nc.vector.memset(S6[:], 1.0)

            if _iters > 1 and _wrap == "all":
                with tc.For_i(0, _iters, 1):
                    whole()
            elif _iters > 1:
                rep("a", stage_a)
                allgather1()
                rep("1", lambda: agg_phase(hw_full, HID_DIM, pp_agg, "agg",
                                           HID_DIM, epi1))
                stats_chain()
                rep("2", phase2)
                allgather2()
                rep("3", lambda: agg_phase(hw2_full, OUT_DIM, pp_w2, "w2",
                                           OUT_DIM, epi3))
            else:
                whole()

            if _skip_compute:
                nc.sync.dma_start(out_d[0:P, :], b2r[:, 0:OUT_DIM])

    nc.compile()
    return nc


_CACHE = {}
_last_in_maps = None
pl = None


def _get_nc(pl_):
    key = (pl_.S, tuple(pl_.T.reshape(-1)),
       tuple(os.environ.get(k, "") for k in (
           "KERNEL_TIME_ITERS", "KERNEL_TIME_WRAP", "KERNEL_TIME_PHASES",
           "KERNEL_SKIP_COMPUTE", "KERNEL_XBUFS", "KERNEL_MBUFS",
           "KERNEL_PSBUFS")))
    if key not in _CACHE:
        _CACHE[key] = _build(pl_)
    return _CACHE[key]


def kernel(h, W1, b1, W2, b2, gamma, beta, src, dst):
    global pl, _last_in_maps
    h = np.asarray(h, np.float32)
    W1 = np.asarray(W1, np.float32)
    W2 = np.asarray(W2, np.float32)
    b2 = np.asarray(b2, np.float32)
    gamma = np.asarray(gamma, np.float32)
    beta = np.asarray(beta, np.float32)
    src = np.asarray(src)
    dst = np.asarray(dst)

    deg_out = np.bincount(src, minlength=N_NODES).astype(np.float64)
    deg_in = np.bincount(dst, minlength=N_NODES).astype(np.float64)
    s_out = (1.0 / np.sqrt(np.maximum(deg_out, 1.0))).astype(np.float32)
    s_in = (1.0 / np.sqrt(np.maximum(deg_in, 1.0))).astype(np.float32)

    pl = _plan(src, dst, s_out, s_in)
    nc = _get_nc(pl)

    # b1 is zero in this problem family and absorbed by BatchNorm anyway
    gmb = np.stack([gamma, beta], axis=1).astype(np.float32)
    b2r = np.tile(b2[None, :], (P, 1)).astype(np.float32)
    iota = np.tile(np.arange(P, dtype=np.float16)[None, :], (P, 1))
    ident = np.eye(P, dtype=np.float32)

    in_maps = []
    for r in range(NCORES):
        # permuted, padded, transposed h in f16
        hp = np.zeros((BP, IN_DIM), np.float16)
        valid = pl.perm[r] >= 0
        hp[valid] = h[pl.perm[r][valid]].astype(np.float16)
        in_maps.append({
            "h": np.ascontiguousarray(hp.T),
            "w1": W1, "w2": W2, "gmb": gmb, "b2r": b2r,
            "sout": pl.sout_t[r],
            "idx": pl.idx16[r],
            "dstloc": pl.dstloc_t[r],
            "sinv": pl.sinv_t[r],
            "iotaf": iota, "identf": ident,
        })
    _last_in_maps = in_maps
    try:
        res = bass_utils.run_bass_kernel_spmd(nc, in_maps, core_ids=list(range(NCORES)))
    except Exception:
        import time as _time
        _time.sleep(130)
        res = bass_utils.run_bass_kernel_spmd(nc, in_maps, core_ids=list(range(NCORES)))
    out = np.zeros((N_NODES, OUT_DIM), np.float32)
    for r in range(NCORES):
        o = res.results[r]["out"]
        valid = pl.perm[r] >= 0
        out[pl.perm[r][valid]] = o[valid]
    return out



# revision 2
# speedup vs baseline: 1.4783x; 1.4783x over previous
"""Self-contained Trainium2 Bass kernel for a 2-layer GCN (GraphConv + BN + ReLU + GraphConv).

v4 strategy (8 NeuronCores, SPMD) — evolves v3 around two observations:
  - dma_gather cost scales with INDEX COUNT, not bytes; ~2.4 ns/row with 4
    SWDGE queues is the floor. Per-edge gathers are unavoidable, so the win
    is removing everything else from the critical path.
  - The layer-1 gather table does NOT need W1 pre-applied: fold the edge
    normalization s_in[dst]*s_out[src] into the per-edge mask weight and
    gather RAW f16 h rows (an ExternalInput — host supplies the permuted
    table). W1 (128x128, square) is applied AFTER aggregation, one extra
    matmul per chunk. This deletes stage A AND the 922us AllGather of the
    hw table entirely.

Structure:
  phase 1: per (octet, quartile): dma_gather h rows; per 128-edge tile one
    DVE mask M[e,d] = (iota[d]==dstloc[e]) * w_e; PE accumulates
    psum[in128, dst128] += X^T M over the chunk's tiles; epilogue: ACT-evac
    to f16, PE matmul W1^T @ agg -> H1[hid, dst], DVE bn_stats per 2 chunks.
  stats: bn_aggr -> (S1,S2) -> AllReduce [128,2] -> A,B affine params.
  phase 2: per chunk: ACT relu(A*H1+B) -> PE (tcn^T @ W2) -> [dst,64] psum
    -> f16 -> DRAM hw2_slice (compact 64-col).
  AG2: AllGather compact table; 4 per-quartile DRAM->DRAM expands to
    256B-row gather layout (separate tensors so quartile-q gathers only
    wait on their own expand).
  phase 3: same gather machinery on the hw2 tables (lhsT sliced to 64
    feats, swapped orientation), + b2 -> out.
"""
import math
import os

import numpy as np

import concourse.bacc as bacc
import concourse.mybir as mybir
import concourse.tile as tile
from concourse import bass_utils

# Problem constants (hardcoded per the task contract).
N_NODES = 100000
N_EDGES = 1600000
IN_DIM = 128
HID_DIM = 128
OUT_DIM = 64
BN_EPS = 1e-5
NCORES = 8
P = 128
C = 98                  # chunks per core
BP = C * P              # padded nodes per core (12544)
B = N_NODES // NCORES   # real nodes per core (12500)
QR = 2 * BP             # table rows per quartile (25088), int16-indexable
OCT = 8                 # chunks per gather octet
NOCT = math.ceil(C / OCT)   # 13
NTAB = NCORES * BP


class Plan:
    pass


def _pack_core(dv, cap):
    """Greedy assignment of nodes (rows of dv [B,4]) to C chunks of <=128
    nodes, respecting per-(chunk, quartile) edge capacities cap [C,4]."""
    tot = dv.sum(1)
    order = np.argsort(-tot)
    loads = np.zeros((C, 4), np.int64)
    counts = np.zeros(C, np.int64)
    assign = np.zeros(len(dv), np.int64)
    for i in order:
        v = dv[i]
        cand = np.nonzero(counts < P)[0]
        newl = loads[cand] + v
        over = np.maximum(newl - cap[cand], 0).sum(1)
        score = over * 100000 + newl.max(1)
        c = cand[np.argmin(score)]
        assign[i] = c
        loads[c] += v
        counts[c] += 1
    return assign, loads, counts


def _swap_repair(dv, assign, loads, cap, max_passes=10):
    """Swap nodes between chunks to push loads under the shared capacities."""
    members = [list(np.nonzero(assign == c)[0]) for c in range(C)]
    for _ in range(max_passes):
        ov = loads - cap
        over = np.argwhere(ov > 0)
        if len(over) == 0:
            break
        over = over[np.argsort(-ov[over[:, 0], over[:, 1]])]
        fixed = 0
        for c, q in over:
            guard = 0
            while loads[c, q] > cap[c, q] and guard < 8:
                guard += 1
                mem = np.array(members[c])
                mem = mem[dv[mem, q] > 0]
                if len(mem) == 0:
                    break
                i = mem[np.argmax(dv[mem, q])]
                vi = dv[i]
                best = None
                room = cap[:, q] - loads[:, q]
                for c2 in np.argsort(-room)[:24]:
                    if c2 == c or room[c2] <= 0:
                        continue
                    for j in members[c2][:48]:
                        vj = dv[j]
                        if vj[q] >= vi[q]:
                            continue
                        nc2 = loads[c2] + vi - vj
                        ncc = loads[c] - vi + vj
                        if (nc2 <= cap[c2]).all() and (ncc <= loads[c]).all():
                            best = (c2, j, nc2, ncc)
                            break
                    if best:
                        break
                if best is None:
                    break
                c2, j, nc2, ncc = best
                members[c].remove(i)
                members[c2].remove(j)
                members[c].append(j)
                members[c2].append(i)
                assign[i], assign[j] = c2, c
                loads[c2], loads[c] = nc2, ncc
                fixed += 1
        if fixed == 0:
            break
    return assign, loads


def _plan(src, dst, h_s_out, s_in_full):
    pl = Plan()
    core = dst // B
    src_core = src // B

    # per-node quartile in-degree vectors (src quartile = src core pair)
    quart_of_src = src_core // 2
    deg = np.zeros((N_NODES, 4), np.int64)
    np.add.at(deg, (dst, quart_of_src), 1)

    # shared target schedule: base 4 tiles (512 edges) per (chunk, quartile);
    # structural per-quartile excess gets extra tiles on the first chunk ranks,
    # identically on every core so the shared schedule stays tight.
    eq = np.zeros((NCORES, 4), np.int64)
    np.add.at(eq, (core, quart_of_src), 1)
    cap = np.full((C, 4), 512, np.int64)
    for q in range(4):
        excess = int(eq[:, q].max()) - C * 512
        extra_tiles = max(0, -(-excess // P)) + 1  # +1 margin
        for j in range(extra_tiles):
            cap[j % C, q] += P * (1 + j // C)

    # per-core balanced chunk assignment + slot permutation
    # slot_of[node] = position in its core's padded 12544-slot table slice
    slot_of = np.zeros(N_NODES, np.int64)
    perm = np.full((NCORES, BP), -1, np.int64)  # slot -> node id (-1 = pad)
    for r in range(NCORES):
        nodes = np.arange(r * B, (r + 1) * B)
        assign, loads, counts = _pack_core(deg[nodes], cap)
        assign, loads = _swap_repair(deg[nodes], assign, loads, cap)
        # place nodes within chunks (chunk ids are shared schedule ranks)
        off = np.zeros(C, np.int64)
        for i, cc in enumerate(assign):
            slot_of[nodes[i]] = cc * P + off[cc]
            perm[r, cc * P + off[cc]] = nodes[i]
            off[cc] += 1
    pl.perm = perm

    tr = np.zeros(N_NODES, np.int64)
    tr[:] = (np.arange(N_NODES) // B) * BP + slot_of
    pl.table_row = tr

    # edge schedule: group = (dst chunk, src quartile)
    e_core = core
    e_chunk = slot_of[dst] // P
    e_dcol = slot_of[dst] % P
    e_q = tr[src] // QR
    cnt = np.zeros((NCORES, C, 4), np.int64)
    np.add.at(cnt, (e_core, e_chunk, e_q), 1)
    T = np.ceil(cnt / P).astype(np.int64).max(axis=0)  # shared [C, 4]
    pl.T = T
    T_total = int(T.sum())
    S = T_total * P
    pl.T_total, pl.S = T_total, S

    # slot offsets: octet-major, quartile, chunk, tile
    goff = np.zeros((C, 4), np.int64)
    region = {}
    tcol = np.zeros((C, 4), np.int64)  # global tile index of (c,q) tile 0
    acc = 0
    tacc = 0
    for o in range(NOCT):
        c0, c1 = o * OCT, min((o + 1) * OCT, C)
        for q in range(4):
            r0 = acc
            for c in range(c0, c1):
                goff[c, q] = acc
                tcol[c, q] = tacc
                acc += T[c, q] * P
                tacc += T[c, q]
            region[(o, q)] = (r0, (acc - r0) // P)
    pl.goff, pl.region, pl.tcol = goff, region, tcol
    assert acc == S

    # per-core slot arrays; edge weight = s_in[dst] * s_out[src] (full
    # GraphConv 'both' normalization folded into the mask)
    srcloc = np.zeros((NCORES, S), np.int16)
    dstloc = np.full((NCORES, S), 999.0, np.float32)
    sinv = np.zeros((NCORES, S), np.float32)

    order = np.lexsort((tr[src], e_q, e_chunk, e_core))
    so, co = src[order], e_core[order]
    cho, qo = e_chunk[order], e_q[order]
    dco = e_dcol[order]
    sio = s_in_full[dst[order]] * h_s_out[src[order]]
    run_sizes = cnt.reshape(-1)
    run_starts = np.concatenate([[0], np.cumsum(run_sizes)])[:-1]
    run_id = (co * C + cho) * 4 + qo
    within = np.arange(len(order)) - run_starts[run_id]
    slots = goff[cho, qo] + within
    srcloc[co, slots] = (tr[so] - qo * QR).astype(np.int16)
    dstloc[co, slots] = dco.astype(np.float32)
    sinv[co, slots] = sio

    # wrapped idx for dma_gather: slot i -> [i % 16, i // 16], replicated x8
    w = srcloc.reshape(NCORES, S // 16, 16)
    pl.idx16 = np.ascontiguousarray(np.tile(w.transpose(0, 2, 1), (1, 8, 1)))
    # [128, T_total] layouts: slot i -> [i % 128, i // 128]
    pl.dstloc_t = np.ascontiguousarray(
        dstloc.reshape(NCORES, T_total, P).transpose(0, 2, 1))
    pl.sinv_t = np.ascontiguousarray(
        sinv.reshape(NCORES, T_total, P).transpose(0, 2, 1))
    return pl


def _build(pl):
    f16, f32 = mybir.dt.float16, mybir.dt.float32
    i16 = mybir.dt.int16
    T, T_total, S = pl.T, pl.T_total, pl.S
    rg = [list(range(NCORES))]

    nc = bacc.Bacc("TRN2", target_bir_lowering=False, debug=False,
                   num_devices=NCORES, num_swdge_queues=4)

    htab_d = nc.dram_tensor("htab", [NTAB, IN_DIM], f16, kind="ExternalInput")
    w1_d = nc.dram_tensor("w1", [IN_DIM, HID_DIM], f16, kind="ExternalInput")
    w2_d = nc.dram_tensor("w2", [HID_DIM, OUT_DIM], f16, kind="ExternalInput")
    gmb_d = nc.dram_tensor("gmb", [HID_DIM, 2], f32, kind="ExternalInput")
    b2r_d = nc.dram_tensor("b2r", [P, OUT_DIM], f32, kind="ExternalInput")
    idx_d = nc.dram_tensor("idx", [P, S // 16], i16, kind="ExternalInput")
    dstloc_d = nc.dram_tensor("dstloc", [P, T_total], f32, kind="ExternalInput")
    sinv_d = nc.dram_tensor("sinv", [P, T_total], f32, kind="ExternalInput")
    iota_d = nc.dram_tensor("iotaf", [P, P], f16, kind="ExternalInput")
    out_d = nc.dram_tensor("out", [BP, OUT_DIM], f32, kind="ExternalOutput")

    hw2_slice = nc.dram_tensor("hw2_slice", [BP, OUT_DIM], f16)
    hw2s_full = nc.dram_tensor("hw2s_full", [NTAB, OUT_DIM], f16, addr_space="Shared")
    hw2q = [nc.dram_tensor(f"hw2q{q}", [QR, P], f16) for q in range(4)]
    stat_in = nc.dram_tensor("stat_in", [P, 2], f32)
    stat_out = nc.dram_tensor("stat_out", [P, 2], f32, addr_space="Shared")

    AF = mybir.ActivationFunctionType
    OP = mybir.AluOpType

    with tile.TileContext(nc) as tc:
        with (
            tc.tile_pool(name="const", bufs=1) as cp,
            tc.tile_pool(name="evac", bufs=4) as ep,
            tc.tile_pool(name="xg", bufs=3) as xp,
            tc.tile_pool(name="mp", bufs=6) as mp,
            tc.tile_pool(name="ps_agg", bufs=3, space="PSUM") as pp_agg,
            tc.tile_pool(name="ps_h1", bufs=2, space="PSUM") as pp_h1,
            tc.tile_pool(name="ps_w2", bufs=3, space="PSUM") as pp_w2,
        ):
            # ---- constants ----
            iota_f = cp.tile([P, P], f16)
            nc.sync.dma_start(iota_f[:], iota_d[:, :])
            w1f = cp.tile([IN_DIM, HID_DIM], f16)
            nc.sync.dma_start(w1f[:], w1_d[:, :])
            w2f = cp.tile([HID_DIM, OUT_DIM], f16)
            nc.sync.dma_start(w2f[:], w2_d[:, :])
            gmb = cp.tile([HID_DIM, 2], f32)
            nc.sync.dma_start(gmb[:], gmb_d[:, :])
            b2r = cp.tile([P, OUT_DIM], f32)
            nc.sync.dma_start(b2r[:], b2r_d[:, :])
            idx_t = cp.tile([P, S // 16], i16)
            nc.sync.dma_start(idx_t[:], idx_d[:, :])
            dl_t = cp.tile([P, T_total], f32)
            nc.sync.dma_start(dl_t[:], dstloc_d[:, :])
            si_t = cp.tile([P, T_total], f32)
            nc.sync.dma_start(si_t[:], sinv_d[:, :])
            H1 = cp.tile([P, BP], f16)
            S6 = cp.tile([P, 49 * 6], f32)

            _iters = int(os.environ.get("KERNEL_TIME_ITERS", "1"))
            _wrap = os.environ.get("KERNEL_TIME_WRAP", "phase")

            # ---- shared agg machinery: one narrow mask + one matmul per tile ----
            _clevel = int(os.environ.get("KERNEL_COMPUTE_LEVEL", "3"))
            if os.environ.get("KERNEL_SKIP_COMPUTE", "0") == "1":
                _clevel = 0
            _skip_compute = _clevel < 3
            _xbufs = int(os.environ.get("KERNEL_XBUFS", "2"))
            _mbufs = int(os.environ.get("KERNEL_MBUFS", "56"))
            _psbufs = int(os.environ.get("KERNEL_PSBUFS", "3"))

            def agg_phase(table_q, lhs_w, psum_pool, psum_tag, swap, epilogue):
                # table_q(q) -> (dram tensor, row offset) for quartile q.
                # Masks are built LOOKAHEAD chunks early so the DVE stays ahead
                # of the PE; epilogue evacs run on ACT, keeping DVE mask builds
                # off the psum-drain critical path.
                LOOKAHEAD = 2
                pending = {}

                def build_masks(c):
                    tiles = []
                    for q in range(4):
                        for t in range(int(T[c, q])):
                            gt = int(pl.tcol[c, q]) + t
                            Mt = mp.tile([P, P], f16, tag="M", bufs=_mbufs)
                            nc.vector.tensor_scalar(
                                out=Mt[:], in0=iota_f[:],
                                scalar1=dl_t[:, gt:gt + 1],
                                scalar2=si_t[:, gt:gt + 1],
                                op0=OP.is_equal, op1=OP.mult,
                            )
                            tiles.append((q, t, Mt))
                    pending[c] = tiles

                for o in range(NOCT):
                    c0, c1 = o * OCT, min((o + 1) * OCT, C)
                    Xq = []
                    for q in range(4):
                        r0, ntiles = pl.region[(o, q)]
                        if ntiles == 0:
                            Xq.append(None)
                            continue
                        tab, roff = table_q(q)
                        X = xp.tile([P, ntiles, HID_DIM], f16, tag=f"Xq{q}", bufs=_xbufs)
                        nc.gpsimd.dma_gather(
                            out_ap=X[:],
                            in_ap=tab[roff:roff + QR, :],
                            idxs_ap=idx_t[:, r0 // 16:r0 // 16 + ntiles * 8],
                            num_idxs=ntiles * P,
                            num_idxs_reg=ntiles * P,
                            elem_size=HID_DIM,
                            single_packet=False,
                            queue_num=q,
                        )
                        Xq.append(X)
                    if _clevel == 0:
                        continue
                    for c in range(c0, c1):
                        if c == 0:
                            for cc in range(min(LOOKAHEAD + 1, C)):
                                build_masks(cc)
                        elif c + LOOKAHEAD < C:
                            build_masks(c + LOOKAHEAD)
                        if _clevel == 1:
                            pending.pop(c)
                            continue
                        Tc = int(T[c].sum())
                        if swap:
                            ps = psum_pool.tile([P, lhs_w], f32, tag=psum_tag,
                                                bufs=_psbufs)
                        else:
                            ps = psum_pool.tile([lhs_w, P], f32, tag=psum_tag,
                                                bufs=_psbufs)
                        ti = 0
                        for q, t, Mt in pending.pop(c):
                            r0, _nt = pl.region[(o, q)]
                            xt0 = int((pl.goff[c, q] - r0) // P)
                            xs = Xq[q][:, xt0 + t, 0:lhs_w]
                            if swap:
                                nc.tensor.matmul(
                                    ps[:], lhsT=Mt[:], rhs=xs,
                                    start=(ti == 0), stop=(ti == Tc - 1),
                                )
                            else:
                                nc.tensor.matmul(
                                    ps[:], lhsT=xs, rhs=Mt[:],
                                    start=(ti == 0), stop=(ti == Tc - 1),
                                )
                            ti += 1
                        if _clevel >= 3:
                            epilogue(c, ps)

            # ---- phase 1: layer-1 aggregation (raw h) -> @W1 -> H1 + bn stats ----
            def epi1(c, ps):
                # evacs on ACT (constant Copy func -> no activation-table reload)
                aggS = ep.tile([P, P], f16, tag="aggS")
                nc.scalar.activation(out=aggS[:], in_=ps[:], func=AF.Copy)
                psH = pp_h1.tile([HID_DIM, P], f32, tag="h1")
                nc.tensor.matmul(psH[:], lhsT=w1f[:], rhs=aggS[:],
                                 start=True, stop=True)
                nc.scalar.activation(out=H1[:, c * P:(c + 1) * P], in_=psH[:],
                                     func=AF.Copy)
                if c % 2 == 1:
                    # equal-count (256-col) records keep bn_aggr's variance
                    # combination exact; C=98 is even so every chunk is covered
                    blk0 = (c // 2) * 2
                    nc.vector.bn_stats(
                        S6[:, (c // 2) * 6:(c // 2) * 6 + 6],
                        H1[:, blk0 * P:(c + 1) * P],
                    )

            # ---- BN stats -> (S1, S2) -> AllReduce -> A, B ----
            ag = cp.tile([P, 2], f32)
            s12 = cp.tile([P, 2], f32)
            msq = cp.tile([P, 1], f32)
            st = cp.tile([P, 2], f32)
            mean = cp.tile([P, 1], f32)
            var = cp.tile([P, 1], f32)
            msq2 = cp.tile([P, 1], f32)
            sd = cp.tile([P, 1], f32)
            inv = cp.tile([P, 1], f32)
            A = cp.tile([P, 1], f32)
            Bb = cp.tile([P, 1], f32)

            def stats_chain():
                nc.vector.bn_aggr(ag[:], S6[:])
                # S1 = mean * BP ; S2 = (var + mean^2) * BP (pad cols are zeros)
                nc.vector.tensor_scalar(out=s12[:, 0:1], in0=ag[:, 0:1],
                                        scalar1=float(BP), scalar2=None, op0=OP.mult)
                nc.vector.tensor_tensor(out=msq[:], in0=ag[:, 0:1], in1=ag[:, 0:1],
                                        op=OP.mult)
                nc.vector.tensor_tensor(out=msq[:], in0=ag[:, 1:2], in1=msq[:], op=OP.add)
                nc.vector.tensor_scalar(out=s12[:, 1:2], in0=msq[:],
                                        scalar1=float(BP), scalar2=None, op0=OP.mult)
                nc.sync.dma_start(stat_in[:, :], s12[:])
                nc.gpsimd.collective_compute(
                    "AllReduce", OP.add, replica_groups=rg,
                    ins=[stat_in.ap().opt()], outs=[stat_out.ap().opt()],
                )
                tc.strict_bb_all_engine_barrier()
                nc.sync.dma_start(st[:], stat_out[:, :])
                nc.vector.tensor_scalar(out=mean[:], in0=st[:, 0:1], scalar1=1.0 / N_NODES,
                                        scalar2=None, op0=OP.mult)
                nc.vector.tensor_scalar(out=var[:], in0=st[:, 1:2], scalar1=1.0 / N_NODES,
                                        scalar2=None, op0=OP.mult)
                nc.vector.tensor_tensor(out=msq2[:], in0=mean[:], in1=mean[:], op=OP.mult)
                nc.vector.tensor_tensor(out=var[:], in0=var[:], in1=msq2[:], op=OP.subtract)
                nc.vector.tensor_scalar(out=sd[:], in0=var[:], scalar1=BN_EPS, scalar2=None,
                                        op0=OP.add)
                nc.scalar.activation(out=sd[:], in_=sd[:], func=AF.Sqrt)
                nc.vector.reciprocal(out=inv[:], in_=sd[:])
                nc.vector.tensor_tensor(out=A[:], in0=inv[:], in1=gmb[:, 0:1], op=OP.mult)
                nc.vector.tensor_tensor(out=Bb[:], in0=mean[:], in1=A[:], op=OP.mult)
                nc.vector.tensor_tensor(out=Bb[:], in0=gmb[:, 1:2], in1=Bb[:], op=OP.subtract)

            # ---- phase 2: hw2 = relu(A*H1+B) @ W2, direct [dst, 64] ----
            def phase2():
                for c in range(C):
                    tcn = ep.tile([P, P], f16, tag="tcn")
                    nc.scalar.activation(out=tcn[:], in_=H1[:, c * P:(c + 1) * P],
                                         func=AF.Relu, bias=Bb[:], scale=A[:])
                    ps2 = pp_w2.tile([P, OUT_DIM], f32, tag="w2")
                    nc.tensor.matmul(ps2[:], lhsT=tcn[:], rhs=w2f[:], start=True, stop=True)
                    hw2c = ep.tile([P, OUT_DIM], f16, tag="hw2c")
                    nc.vector.tensor_copy(hw2c[:], ps2[:])
                    nc.sync.dma_start(hw2_slice[c * P:(c + 1) * P, :], hw2c[:, :])

            def allgather2():
                # gather the compact 64-col table (half the bytes), then four
                # per-quartile DRAM->DRAM DMAs expand to the 256B-row gather
                # layout; separate tensors let quartile-q gathers start as
                # soon as their own expand lands.
                nc.gpsimd.collective_compute(
                    "AllGather", OP.bypass, replica_groups=rg,
                    ins=[hw2_slice.ap().opt()], outs=[hw2s_full.ap().opt()],
                )
                tc.strict_bb_all_engine_barrier()
                for q in range(4):
                    nc.sync.dma_start(hw2q[q][:, 0:OUT_DIM],
                                      hw2s_full[q * QR:(q + 1) * QR, :])

            # ---- phase 3: layer-2 aggregation + b2 -> out ----
            # swapped matmul orientation: psum arrives [dst128, feat64]
            def epi3(c, ps):
                oc = ep.tile([P, OUT_DIM], f32, tag="oc")
                nc.vector.tensor_tensor(out=oc[:], in0=ps[:], in1=b2r[:], op=OP.add)
                nc.sync.dma_start(out_d[c * P:(c + 1) * P, :], oc[:, :])

            def tab1(q):
                return htab_d, q * QR

            def tab3(q):
                return hw2q[q], 0

            def whole():
                agg_phase(tab1, HID_DIM, pp_agg, "agg", False, epi1)
                stats_chain()
                phase2()
                allgather2()
                agg_phase(tab3, OUT_DIM, pp_w2, "w2", True, epi3)

            _phases = set(os.environ.get("KERNEL_TIME_PHASES", "1,2,3").split(","))

            def rep(section, fn):
                if _iters > 1 and section in _phases:
                    with tc.For_i(0, _iters, 1):
                        fn()
                else:
                    fn()

            if _skip_compute:
                nc.vector.memset(H1[:], 0.0)
                nc.vector.memset(S6[:], 1.0)

            if _iters > 1 and _wrap == "all":
                with tc.For_i(0, _iters, 1):
                    whole()
            elif _iters > 1:
                rep("1", lambda: agg_phase(tab1, HID_DIM, pp_agg, "agg",
                                           False, epi1))
                stats_chain()
                rep("2", phase2)
                allgather2()
                rep("3", lambda: agg_phase(tab3, OUT_DIM, pp_w2, "w2",
                                           True, epi3))
            else:
                whole()

            if _skip_compute:
                nc.sync.dma_start(out_d[0:P, :], b2r[:, 0:OUT_DIM])

    nc.compile()
    return nc


_CACHE = {}
_last_in_maps = None
pl = None


def _get_nc(pl_):
    key = (pl_.S, tuple(pl_.T.reshape(-1)),
       tuple(os.environ.get(k, "") for k in (
           "KERNEL_TIME_ITERS", "KERNEL_TIME_WRAP", "KERNEL_TIME_PHASES",
           "KERNEL_SKIP_COMPUTE", "KERNEL_COMPUTE_LEVEL", "KERNEL_XBUFS",
           "KERNEL_MBUFS", "KERNEL_PSBUFS")))
    if key not in _CACHE:
        _CACHE[key] = _build(pl_)
    return _CACHE[key]


def kernel(h, W1, b1, W2, b2, gamma, beta, src, dst):
    global pl, _last_in_maps
    h = np.asarray(h, np.float32)
    W1 = np.asarray(W1, np.float32)
    W2 = np.asarray(W2, np.float32)
    b2 = np.asarray(b2, np.float32)
    gamma = np.asarray(gamma, np.float32)
    beta = np.asarray(beta, np.float32)
    src = np.asarray(src)
    dst = np.asarray(dst)

    deg_out = np.bincount(src, minlength=N_NODES).astype(np.float64)
    deg_in = np.bincount(dst, minlength=N_NODES).astype(np.float64)
    s_out = (1.0 / np.sqrt(np.maximum(deg_out, 1.0))).astype(np.float32)
    s_in = (1.0 / np.sqrt(np.maximum(deg_in, 1.0))).astype(np.float32)

    pl = _plan(src, dst, s_out, s_in)
    nc = _get_nc(pl)

    # b1 is zero in this problem family and absorbed by BatchNorm anyway
    gmb = np.stack([gamma, beta], axis=1).astype(np.float32)
    b2r = np.tile(b2[None, :], (P, 1)).astype(np.float32)
    iota = np.tile(np.arange(P, dtype=np.float16)[None, :], (P, 1))

    # full permuted h table (identical for every core): row tr[n] = h[n]
    htab = np.zeros((NTAB, IN_DIM), np.float16)
    for r in range(NCORES):
        valid = pl.perm[r] >= 0
        htab[r * BP + np.nonzero(valid)[0]] = h[pl.perm[r][valid]].astype(np.float16)

    in_maps = []
    for r in range(NCORES):
        in_maps.append({
            "htab": htab,
            "w1": W1.astype(np.float16), "w2": W2.astype(np.float16),
            "gmb": gmb, "b2r": b2r,
            "idx": pl.idx16[r],
            "dstloc": pl.dstloc_t[r],
            "sinv": pl.sinv_t[r],
            "iotaf": iota,
        })
    _last_in_maps = in_maps
    try:
        res = bass_utils.run_bass_kernel_spmd(nc, in_maps, core_ids=list(range(NCORES)))
    except Exception:
        import time as _time
        _time.sleep(130)
        res = bass_utils.run_bass_kernel_spmd(nc, in_maps, core_ids=list(range(NCORES)))
    out = np.zeros((N_NODES, OUT_DIM), np.float32)
    for r in range(NCORES):
        o = res.results[r]["out"]
        valid = pl.perm[r] >= 0
        out[pl.perm[r][valid]] = o[valid]
    return out


# revision 11
# speedup vs baseline: 2.4275x; 1.6420x over previous
"""Self-contained Trainium2 Bass kernel for a 2-layer GCN (GraphConv + BN + ReLU + GraphConv).

v4 strategy (8 NeuronCores, SPMD) — evolves v3 around two observations:
  - dma_gather cost scales with INDEX COUNT, not bytes; ~2.4 ns/row with 4
    SWDGE queues is the floor. Per-edge gathers are unavoidable, so the win
    is removing everything else from the critical path.
  - The layer-1 gather table does NOT need W1 pre-applied: fold the edge
    normalization s_in[dst]*s_out[src] into the per-edge mask weight and
    gather RAW f16 h rows (an ExternalInput — host supplies the permuted
    table). W1 (128x128, square) is applied AFTER aggregation, one extra
    matmul per chunk. This deletes stage A AND the 922us AllGather of the
    hw table entirely.

Structure:
  phase 1: per (octet, quartile): dma_gather h rows; masks
    M[e,d] = (d==dstloc[e]) * w_e are HOST-precomputed and DMA-streamed per
    octet (contiguous loads; the DVE builds nothing during gather phases —
    measured: DVE mask building adds ~340us/phase of serialization). PE
    accumulates psum[in128, dst128] += X^T M over the chunk's tiles;
    epilogue: ACT-evac to f16, PE matmul W1^T @ agg -> H1[hid, dst],
    DVE bn_stats per 2 chunks. The SAME mask table drives both layers.
  stats: bn_aggr -> (S1,S2) -> AllReduce [128,2] -> A,B affine params.
  phase 2: per chunk: ACT relu(A*H1+B) -> PE (tcn^T @ W2) -> [dst,64] psum
    -> f16 -> DRAM hw2_slice (compact 64-col).
  AG2: AllGather compact table; 4 per-quartile DRAM->DRAM expands to
    256B-row gather layout (separate tensors so quartile-q gathers only
    wait on their own expand).
  phase 3: same gather machinery on the hw2 tables (lhsT sliced to 64
    feats, swapped orientation), + b2 -> out.
"""
import math
import os

import numpy as np

import concourse.bacc as bacc
import concourse.mybir as mybir
import concourse.tile as tile
from concourse import bass_utils

# Problem constants (hardcoded per the task contract).
N_NODES = 100000
N_EDGES = 1600000
IN_DIM = 128
HID_DIM = 128
OUT_DIM = 64
BN_EPS = 1e-5
NCORES = 8
P = 128
C = 98                  # chunks per core
BP = C * P              # padded nodes per core (12544)
B = N_NODES // NCORES   # real nodes per core (12500)
QR = 2 * BP             # table rows per quartile (25088), int16-indexable
OCT = 8                 # chunks per gather octet
NOCT = math.ceil(C / OCT)   # 13
NTAB = NCORES * BP


class Plan:
    pass


def _pack_core(dv, cap):
    """Greedy assignment of nodes (rows of dv [B,4]) to C chunks of <=128
    nodes, respecting per-(chunk, quartile) edge capacities cap [C,4]."""
    tot = dv.sum(1)
    order = np.argsort(-tot)
    loads = np.zeros((C, 4), np.int64)
    counts = np.zeros(C, np.int64)
    assign = np.zeros(len(dv), np.int64)
    for i in order:
        v = dv[i]
        cand = np.nonzero(counts < P)[0]
        newl = loads[cand] + v
        over = np.maximum(newl - cap[cand], 0).sum(1)
        score = over * 100000 + newl.max(1)
        c = cand[np.argmin(score)]
        assign[i] = c
        loads[c] += v
        counts[c] += 1
    return assign, loads, counts


def _swap_repair(dv, assign, loads, cap, max_passes=10):
    """Swap nodes between chunks to push loads under the shared capacities."""
    members = [list(np.nonzero(assign == c)[0]) for c in range(C)]
    for _ in range(max_passes):
        ov = loads - cap
        over = np.argwhere(ov > 0)
        if len(over) == 0:
            break
        over = over[np.argsort(-ov[over[:, 0], over[:, 1]])]
        fixed = 0
        for c, q in over:
            guard = 0
            while loads[c, q] > cap[c, q] and guard < 8:
                guard += 1
                mem = np.array(members[c])
                mem = mem[dv[mem, q] > 0]
                if len(mem) == 0:
                    break
                i = mem[np.argmax(dv[mem, q])]
                vi = dv[i]
                best = None
                room = cap[:, q] - loads[:, q]
                for c2 in np.argsort(-room)[:24]:
                    if c2 == c or room[c2] <= 0:
                        continue
                    for j in members[c2][:48]:
                        vj = dv[j]
                        if vj[q] >= vi[q]:
                            continue
                        nc2 = loads[c2] + vi - vj
                        ncc = loads[c] - vi + vj
                        if (nc2 <= cap[c2]).all() and (ncc <= loads[c]).all():
                            best = (c2, j, nc2, ncc)
                            break
                    if best:
                        break
                if best is None:
                    break
                c2, j, nc2, ncc = best
                members[c].remove(i)
                members[c2].remove(j)
                members[c].append(j)
                members[c2].append(i)
                assign[i], assign[j] = c2, c
                loads[c2], loads[c] = nc2, ncc
                fixed += 1
        if fixed == 0:
            break
    return assign, loads


def _plan(src, dst, h_s_out, s_in_full):
    pl = Plan()
    core = dst // B
    src_core = src // B

    # per-node quartile in-degree vectors (src quartile = src core pair)
    quart_of_src = src_core // 2
    deg = np.zeros((N_NODES, 4), np.int64)
    np.add.at(deg, (dst, quart_of_src), 1)

    # shared target schedule: base 4 tiles (512 edges) per (chunk, quartile);
    # structural per-quartile excess gets extra tiles on the first chunk ranks,
    # identically on every core so the shared schedule stays tight.
    eq = np.zeros((NCORES, 4), np.int64)
    np.add.at(eq, (core, quart_of_src), 1)
    cap = np.full((C, 4), 512, np.int64)
    for q in range(4):
        excess = int(eq[:, q].max()) - C * 512
        extra_tiles = max(0, -(-excess // P)) + 1  # +1 margin
        for j in range(extra_tiles):
            cap[j % C, q] += P * (1 + j // C)

    # per-core balanced chunk assignment + slot permutation
    # slot_of[node] = position in its core's padded 12544-slot table slice
    slot_of = np.zeros(N_NODES, np.int64)
    perm = np.full((NCORES, BP), -1, np.int64)  # slot -> node id (-1 = pad)
    for r in range(NCORES):
        nodes = np.arange(r * B, (r + 1) * B)
        assign, loads, counts = _pack_core(deg[nodes], cap)
        assign, loads = _swap_repair(deg[nodes], assign, loads, cap)
        # place nodes within chunks (chunk ids are shared schedule ranks)
        off = np.zeros(C, np.int64)
        for i, cc in enumerate(assign):
            slot_of[nodes[i]] = cc * P + off[cc]
            perm[r, cc * P + off[cc]] = nodes[i]
            off[cc] += 1
    pl.perm = perm

    tr = np.zeros(N_NODES, np.int64)
    tr[:] = (np.arange(N_NODES) // B) * BP + slot_of
    pl.table_row = tr

    # edge schedule: group = (dst chunk, src quartile)
    e_core = core
    e_chunk = slot_of[dst] // P
    e_dcol = slot_of[dst] % P
    e_q = tr[src] // QR
    cnt = np.zeros((NCORES, C, 4), np.int64)
    np.add.at(cnt, (e_core, e_chunk, e_q), 1)
    T = np.ceil(cnt / P).astype(np.int64).max(axis=0)  # shared [C, 4]
    pl.T = T
    T_total = int(T.sum())
    S = T_total * P
    pl.T_total, pl.S = T_total, S

    # slot offsets: octet-major, quartile, chunk, tile
    goff = np.zeros((C, 4), np.int64)
    region = {}
    tcol = np.zeros((C, 4), np.int64)  # global tile index of (c,q) tile 0
    oct_trange = []
    acc = 0
    tacc = 0
    for o in range(NOCT):
        c0, c1 = o * OCT, min((o + 1) * OCT, C)
        t0_o = tacc
        for q in range(4):
            r0 = acc
            for c in range(c0, c1):
                goff[c, q] = acc
                tcol[c, q] = tacc
                acc += T[c, q] * P
                tacc += T[c, q]
            region[(o, q)] = (r0, (acc - r0) // P)
        oct_trange.append((t0_o, tacc))
    pl.goff, pl.region, pl.tcol = goff, region, tcol
    pl.oct_trange = oct_trange
    assert acc == S

    # per-core slot arrays; edge weight = s_in[dst] * s_out[src] (full
    # GraphConv 'both' normalization folded into the mask)
    srcloc = np.zeros((NCORES, S), np.int16)
    dstloc = np.full((NCORES, S), 999.0, np.float32)
    sinv = np.zeros((NCORES, S), np.float32)

    order = np.lexsort((tr[src], e_q, e_chunk, e_core))
    so, co = src[order], e_core[order]
    cho, qo = e_chunk[order], e_q[order]
    dco = e_dcol[order]
    sio = s_in_full[dst[order]] * h_s_out[src[order]]
    run_sizes = cnt.reshape(-1)
    run_starts = np.concatenate([[0], np.cumsum(run_sizes)])[:-1]
    run_id = (co * C + cho) * 4 + qo
    within = np.arange(len(order)) - run_starts[run_id]
    slots = goff[cho, qo] + within
    srcloc[co, slots] = (tr[so] - qo * QR).astype(np.int16)
    dstloc[co, slots] = dco.astype(np.float32)
    sinv[co, slots] = sio

    # wrapped idx for dma_gather: slot i -> [i % 16, i // 16], replicated x8
    w = srcloc.reshape(NCORES, S // 16, 16)
    pl.idx16 = np.ascontiguousarray(np.tile(w.transpose(0, 2, 1), (1, 8, 1)))
    # host-built mask table [P, T_total*P] f16 per core:
    # tile gt covers slots [gt*128, (gt+1)*128); mask[s%128, gt*128 + dstloc[s]]
    # = w_e[s]. DMA'd per octet on device (no DVE mask building at all).
    pl.mask_t = np.zeros((NCORES, P, T_total * P), np.float16)
    for r in range(NCORES):
        sl = np.nonzero(dstloc[r] < 999.0)[0]
        rows = sl % P
        cols = (sl // P) * P + dstloc[r, sl].astype(np.int64)
        pl.mask_t[r, rows, cols] = sinv[r, sl].astype(np.float16)
    return pl


def _build(pl):
    f16, f32 = mybir.dt.float16, mybir.dt.float32
    i16 = mybir.dt.int16
    T, T_total, S = pl.T, pl.T_total, pl.S
    rg = [list(range(NCORES))]

    nc = bacc.Bacc("TRN2", target_bir_lowering=False, debug=False,
                   num_devices=NCORES, num_swdge_queues=4)

    htab_d = nc.dram_tensor("htab", [NTAB, IN_DIM], f16, kind="ExternalInput")
    w1_d = nc.dram_tensor("w1", [IN_DIM, HID_DIM], f16, kind="ExternalInput")
    w2_d = nc.dram_tensor("w2", [HID_DIM, OUT_DIM], f16, kind="ExternalInput")
    gmb_d = nc.dram_tensor("gmb", [HID_DIM, 2], f32, kind="ExternalInput")
    b2r_d = nc.dram_tensor("b2r", [P, OUT_DIM], f32, kind="ExternalInput")
    idx_d = nc.dram_tensor("idx", [P, S // 16], i16, kind="ExternalInput")
    mask_d = nc.dram_tensor("maskt", [P, T_total * P], f16, kind="ExternalInput")
    out_d = nc.dram_tensor("out", [BP, OUT_DIM], f32, kind="ExternalOutput")

    hw2_slice = nc.dram_tensor("hw2_slice", [BP, OUT_DIM], f16)
    hw2s_full = nc.dram_tensor("hw2s_full", [NTAB, OUT_DIM], f16, addr_space="Shared")
    hw2q = [nc.dram_tensor(f"hw2q{q}", [QR, P], f16) for q in range(4)]
    stat_in = nc.dram_tensor("stat_in", [P, 2], f32)
    stat_out = nc.dram_tensor("stat_out", [P, 2], f32, addr_space="Shared")

    AF = mybir.ActivationFunctionType
    OP = mybir.AluOpType

    with tile.TileContext(nc) as tc:
        with (
            tc.tile_pool(name="const", bufs=1) as cp,
            tc.tile_pool(name="evac", bufs=4) as ep,
            tc.tile_pool(name="xg", bufs=3) as xp,
            tc.tile_pool(name="mp", bufs=6) as mp,
            tc.tile_pool(name="ps_agg", bufs=3, space="PSUM") as pp_agg,
            tc.tile_pool(name="ps_h1", bufs=2, space="PSUM") as pp_h1,
            tc.tile_pool(name="ps_w2", bufs=3, space="PSUM") as pp_w2,
        ):
            # ---- constants ----
            w1f = cp.tile([IN_DIM, HID_DIM], f16)
            nc.sync.dma_start(w1f[:], w1_d[:, :])
            w2f = cp.tile([HID_DIM, OUT_DIM], f16)
            nc.sync.dma_start(w2f[:], w2_d[:, :])
            gmb = cp.tile([HID_DIM, 2], f32)
            nc.sync.dma_start(gmb[:], gmb_d[:, :])
            b2r = cp.tile([P, OUT_DIM], f32)
            nc.sync.dma_start(b2r[:], b2r_d[:, :])
            idx_t = cp.tile([P, S // 16], i16)
            nc.sync.dma_start(idx_t[:], idx_d[:, :])
            H1 = cp.tile([P, BP], f16)
            S6 = cp.tile([P, 49 * 6], f32)

            _iters = int(os.environ.get("KERNEL_TIME_ITERS", "1"))
            _wrap = os.environ.get("KERNEL_TIME_WRAP", "phase")

            # ---- shared agg machinery: one narrow mask + one matmul per tile ----
            _clevel = int(os.environ.get("KERNEL_COMPUTE_LEVEL", "3"))
            if os.environ.get("KERNEL_SKIP_COMPUTE", "0") == "1":
                _clevel = 0
            _skip_compute = _clevel < 3
            _xbufs = int(os.environ.get("KERNEL_XBUFS", "2"))
            _mbufs = int(os.environ.get("KERNEL_MBUFS", "2"))
            _psbufs = int(os.environ.get("KERNEL_PSBUFS", "3"))

            def load_masks(o):
                gt0, gt1 = pl.oct_trange[o]
                Mo = mp.tile([P, gt1 - gt0, P], f16, tag="M", bufs=_mbufs)
                nc.sync.dma_start(Mo[:], mask_d[:, gt0 * P:gt1 * P])
                return Mo

            def agg_phase(table_q, lhs_w, psum_pool, psum_tag, swap, epilogue,
                          prefetch=None):
                # table_q(q) -> (dram tensor, row offset) for quartile q.
                # Masks are host-precomputed and DMA-streamed per octet (the
                # DVE does nothing here); epilogue evacs run on ACT.
                for o in range(NOCT):
                    c0, c1 = o * OCT, min((o + 1) * OCT, C)
                    gt0, _ = pl.oct_trange[o]
                    if prefetch and o in prefetch:
                        Mo = prefetch[o]
                    elif _clevel >= 1:
                        Mo = load_masks(o)
                    Xq = []
                    for q in range(4):
                        r0, ntiles = pl.region[(o, q)]
                        if ntiles == 0:
                            Xq.append(None)
                            continue
                        tab, roff = table_q(q)
                        X = xp.tile([P, ntiles, HID_DIM], f16, tag=f"Xq{q}", bufs=_xbufs)
                        nc.gpsimd.dma_gather(
                            out_ap=X[:],
                            in_ap=tab[roff:roff + QR, :],
                            idxs_ap=idx_t[:, r0 // 16:r0 // 16 + ntiles * 8],
                            num_idxs=ntiles * P,
                            num_idxs_reg=ntiles * P,
                            elem_size=HID_DIM,
                            single_packet=False,
                            queue_num=q,
                        )
                        Xq.append(X)
                    if _clevel <= 1:
                        continue
                    for c in range(c0, c1):
                        Tc = int(T[c].sum())
                        if swap:
                            ps = psum_pool.tile([P, lhs_w], f32, tag=psum_tag,
                                                bufs=_psbufs)
                        else:
                            ps = psum_pool.tile([lhs_w, P], f32, tag=psum_tag,
                                                bufs=_psbufs)
                        ti = 0
                        for q in range(4):
                            r0, _nt = pl.region[(o, q)]
                            xt0 = int((pl.goff[c, q] - r0) // P)
                            for t in range(int(T[c, q])):
                                gt = int(pl.tcol[c, q]) + t
                                Mt = Mo[:, gt - gt0, :]
                                xs = Xq[q][:, xt0 + t, 0:lhs_w]
                                if swap:
                                    nc.tensor.matmul(
                                        ps[:], lhsT=Mt, rhs=xs,
                                        start=(ti == 0), stop=(ti == Tc - 1),
                                    )
                                else:
                                    nc.tensor.matmul(
                                        ps[:], lhsT=xs, rhs=Mt,
                                        start=(ti == 0), stop=(ti == Tc - 1),
                                    )
                                ti += 1
                        if _clevel >= 3:
                            epilogue(c, ps)

            # ---- phase 1: layer-1 aggregation (raw h) -> @W1 -> H1 + bn stats ----
            def epi1(c, ps):
                # evacs on ACT (constant Copy func -> no activation-table reload)
                aggS = ep.tile([P, P], f16, tag="aggS")
                nc.scalar.activation(out=aggS[:], in_=ps[:], func=AF.Copy)
                psH = pp_h1.tile([HID_DIM, P], f32, tag="h1")
                nc.tensor.matmul(psH[:], lhsT=w1f[:], rhs=aggS[:],
                                 start=True, stop=True)
                nc.scalar.activation(out=H1[:, c * P:(c + 1) * P], in_=psH[:],
                                     func=AF.Copy)
                if c % 2 == 1:
                    # equal-count (256-col) records keep bn_aggr's variance
                    # combination exact; C=98 is even so every chunk is covered
                    blk0 = (c // 2) * 2
                    nc.vector.bn_stats(
                        S6[:, (c // 2) * 6:(c // 2) * 6 + 6],
                        H1[:, blk0 * P:(c + 1) * P],
                    )

            # ---- BN stats -> (S1, S2) -> AllReduce -> A, B ----
            ag = cp.tile([P, 2], f32)
            s12 = cp.tile([P, 2], f32)
            msq = cp.tile([P, 1], f32)
            st = cp.tile([P, 2], f32)
            mean = cp.tile([P, 1], f32)
            var = cp.tile([P, 1], f32)
            msq2 = cp.tile([P, 1], f32)
            sd = cp.tile([P, 1], f32)
            inv = cp.tile([P, 1], f32)
            A = cp.tile([P, 1], f32)
            Bb = cp.tile([P, 1], f32)

            def stats_chain():
                nc.vector.bn_aggr(ag[:], S6[:])
                # S1 = mean * BP ; S2 = (var + mean^2) * BP (pad cols are zeros)
                nc.vector.tensor_scalar(out=s12[:, 0:1], in0=ag[:, 0:1],
                                        scalar1=float(BP), scalar2=None, op0=OP.mult)
                nc.vector.tensor_tensor(out=msq[:], in0=ag[:, 0:1], in1=ag[:, 0:1],
                                        op=OP.mult)
                nc.vector.tensor_tensor(out=msq[:], in0=ag[:, 1:2], in1=msq[:], op=OP.add)
                nc.vector.tensor_scalar(out=s12[:, 1:2], in0=msq[:],
                                        scalar1=float(BP), scalar2=None, op0=OP.mult)
                nc.sync.dma_start(stat_in[:, :], s12[:])
                nc.gpsimd.collective_compute(
                    "AllReduce", OP.add, replica_groups=rg,
                    ins=[stat_in.ap().opt()], outs=[stat_out.ap().opt()],
                )
                tc.strict_bb_all_engine_barrier()
                nc.sync.dma_start(st[:], stat_out[:, :])
                nc.vector.tensor_scalar(out=mean[:], in0=st[:, 0:1], scalar1=1.0 / N_NODES,
                                        scalar2=None, op0=OP.mult)
                nc.vector.tensor_scalar(out=var[:], in0=st[:, 1:2], scalar1=1.0 / N_NODES,
                                        scalar2=None, op0=OP.mult)
                nc.vector.tensor_tensor(out=msq2[:], in0=mean[:], in1=mean[:], op=OP.mult)
                nc.vector.tensor_tensor(out=var[:], in0=var[:], in1=msq2[:], op=OP.subtract)
                nc.vector.tensor_scalar(out=sd[:], in0=var[:], scalar1=BN_EPS, scalar2=None,
                                        op0=OP.add)
                nc.scalar.activation(out=sd[:], in_=sd[:], func=AF.Sqrt)
                nc.vector.reciprocal(out=inv[:], in_=sd[:])
                nc.vector.tensor_tensor(out=A[:], in0=inv[:], in1=gmb[:, 0:1], op=OP.mult)
                nc.vector.tensor_tensor(out=Bb[:], in0=mean[:], in1=A[:], op=OP.mult)
                nc.vector.tensor_tensor(out=Bb[:], in0=gmb[:, 1:2], in1=Bb[:], op=OP.subtract)

            # ---- phase 2: hw2 = relu(A*H1+B) @ W2, direct [dst, 64] ----
            def phase2():
                for c in range(C):
                    tcn = ep.tile([P, P], f16, tag="tcn")
                    nc.scalar.activation(out=tcn[:], in_=H1[:, c * P:(c + 1) * P],
                                         func=AF.Relu, bias=Bb[:], scale=A[:])
                    ps2 = pp_w2.tile([P, OUT_DIM], f32, tag="w2")
                    nc.tensor.matmul(ps2[:], lhsT=tcn[:], rhs=w2f[:], start=True, stop=True)
                    hw2c = ep.tile([P, OUT_DIM], f16, tag="hw2c")
                    nc.vector.tensor_copy(hw2c[:], ps2[:])
                    nc.sync.dma_start(hw2_slice[c * P:(c + 1) * P, :], hw2c[:, :])

            def allgather2():
                # gather the compact 64-col table (half the bytes), then four
                # per-quartile DRAM->DRAM DMAs expand to the 256B-row gather
                # layout; separate tensors let quartile-q gathers start as
                # soon as their own expand lands.
                nc.gpsimd.collective_compute(
                    "AllGather", OP.bypass, replica_groups=rg,
                    ins=[hw2_slice.ap().opt()], outs=[hw2s_full.ap().opt()],
                )
                tc.strict_bb_all_engine_barrier()
                for q in range(4):
                    nc.sync.dma_start(hw2q[q][:, 0:OUT_DIM],
                                      hw2s_full[q * QR:(q + 1) * QR, :])

            # ---- phase 3: layer-2 aggregation + b2 -> out ----
            # swapped matmul orientation: psum arrives [dst128, feat64]
            def epi3(c, ps):
                oc = ep.tile([P, OUT_DIM], f32, tag="oc")
                nc.vector.tensor_tensor(out=oc[:], in0=ps[:], in1=b2r[:], op=OP.add)
                nc.sync.dma_start(out_d[c * P:(c + 1) * P, :], oc[:, :])

            def tab1(q):
                return htab_d, q * QR

            def tab3(q):
                return hw2q[q], 0

            def whole():
                agg_phase(tab1, HID_DIM, pp_agg, "agg", False, epi1)
                stats_chain()
                phase2()
                # prefetch first phase-3 mask octets; the DMAs land during
                # the AllGather window
                pf = {o: load_masks(o) for o in range(2)} if _clevel >= 1 else None
                allgather2()
                agg_phase(tab3, OUT_DIM, pp_w2, "w2", True, epi3, prefetch=pf)

            _phases = set(os.environ.get("KERNEL_TIME_PHASES", "1,2,3").split(","))

            def rep(section, fn):
                if _iters > 1 and section in _phases:
                    with tc.For_i(0, _iters, 1):
                        fn()
                else:
                    fn()

            if _skip_compute:
                nc.vector.memset(H1[:], 0.0)
                nc.vector.memset(S6[:], 1.0)

            if _iters > 1 and _wrap == "all":
                with tc.For_i(0, _iters, 1):
                    whole()
            elif _iters > 1:
                rep("1", lambda: agg_phase(tab1, HID_DIM, pp_agg, "agg",
                                           False, epi1))
                stats_chain()
                rep("2", phase2)
                allgather2()
                rep("3", lambda: agg_phase(tab3, OUT_DIM, pp_w2, "w2",
                                           True, epi3))
            else:
                whole()

            if _skip_compute:
                nc.sync.dma_start(out_d[0:P, :], b2r[:, 0:OUT_DIM])

    nc.compile()
    return nc


_CACHE = {}
_last_in_maps = None
pl = None


def _get_nc(pl_):
    key = (pl_.S, tuple(pl_.T.reshape(-1)),
       tuple(os.environ.get(k, "") for k in (
           "KERNEL_TIME_ITERS", "KERNEL_TIME_WRAP", "KERNEL_TIME_PHASES",
           "KERNEL_SKIP_COMPUTE", "KERNEL_COMPUTE_LEVEL", "KERNEL_XBUFS",
           "KERNEL_MBUFS", "KERNEL_PSBUFS")))
    if key not in _CACHE:
        _CACHE[key] = _build(pl_)
    return _CACHE[key]


def kernel(h, W1, b1, W2, b2, gamma, beta, src, dst):
    global pl, _last_in_maps
    h = np.asarray(h, np.float32)
    W1 = np.asarray(W1, np.float32)
    W2 = np.asarray(W2, np.float32)
    b2 = np.asarray(b2, np.float32)
    gamma = np.asarray(gamma, np.float32)
    beta = np.asarray(beta, np.float32)
    src = np.asarray(src)
    dst = np.asarray(dst)

    deg_out = np.bincount(src, minlength=N_NODES).astype(np.float64)
    deg_in = np.bincount(dst, minlength=N_NODES).astype(np.float64)
    s_out = (1.0 / np.sqrt(np.maximum(deg_out, 1.0))).astype(np.float32)
    s_in = (1.0 / np.sqrt(np.maximum(deg_in, 1.0))).astype(np.float32)

    pl = _plan(src, dst, s_out, s_in)
    nc = _get_nc(pl)

    # b1 is zero in this problem family and absorbed by BatchNorm anyway
    gmb = np.stack([gamma, beta], axis=1).astype(np.float32)
    b2r = np.tile(b2[None, :], (P, 1)).astype(np.float32)

    # full permuted h table (identical for every core): row tr[n] = h[n]
    htab = np.zeros((NTAB, IN_DIM), np.float16)
    for r in range(NCORES):
        valid = pl.perm[r] >= 0
        htab[r * BP + np.nonzero(valid)[0]] = h[pl.perm[r][valid]].astype(np.float16)

    in_maps = []
    for r in range(NCORES):
        in_maps.append({
            "htab": htab,
            "w1": W1.astype(np.float16), "w2": W2.astype(np.float16),
            "gmb": gmb, "b2r": b2r,
            "idx": pl.idx16[r],
            "maskt": pl.mask_t[r],
        })
    _last_in_maps = in_maps
    try:
        res = bass_utils.run_bass_kernel_spmd(nc, in_maps, core_ids=list(range(NCORES)))
    except Exception:
        import time as _time
        _time.sleep(130)
        res = bass_utils.run_bass_kernel_spmd(nc, in_maps, core_ids=list(range(NCORES)))
    out = np.zeros((N_NODES, OUT_DIM), np.float32)
    for r in range(NCORES):
        o = res.results[r]["out"]
        valid = pl.perm[r] >= 0
        out[pl.perm[r][valid]] = o[valid]
    return out


# revision 18
# speedup vs baseline: 2.5177x; 1.0372x over previous
"""Self-contained Trainium2 Bass kernel for a 2-layer GCN (GraphConv + BN + ReLU + GraphConv).

v4 strategy (8 NeuronCores, SPMD) — evolves v3 around two observations:
  - dma_gather cost scales with INDEX COUNT, not bytes; ~2.4 ns/row with 4
    SWDGE queues is the floor. Per-edge gathers are unavoidable, so the win
    is removing everything else from the critical path.
  - The layer-1 gather table does NOT need W1 pre-applied: fold the edge
    normalization s_in[dst]*s_out[src] into the per-edge mask weight and
    gather RAW f16 h rows (an ExternalInput — host supplies the permuted
    table). W1 (128x128, square) is applied AFTER aggregation, one extra
    matmul per chunk. This deletes stage A AND the 922us AllGather of the
    hw table entirely.

Structure:
  phase 1: per (octet, quartile): dma_gather h rows; masks
    M[e,d] = (d==dstloc[e]) * w_e are HOST-precomputed and DMA-streamed per
    octet (contiguous loads; the DVE builds nothing during gather phases —
    measured: DVE mask building adds ~340us/phase of serialization). PE
    accumulates psum[in128, dst128] += X^T M over the chunk's tiles;
    epilogue: ACT-evac to f16, PE matmul W1^T @ agg -> H1[hid, dst],
    DVE bn_stats per 2 chunks. The SAME mask table drives both layers.
  stats: bn_aggr -> (S1,S2) -> AllReduce [128,2] -> A,B affine params.
  phase 2: per chunk: ACT relu(A*H1+B) -> PE (tcn^T @ W2) -> [dst,64] psum
    -> f16 -> DRAM hw2_slice (compact 64-col).
  AG2: AllGather compact table; 4 per-quartile DRAM->DRAM expands to
    256B-row gather layout (separate tensors so quartile-q gathers only
    wait on their own expand).
  phase 3: same gather machinery on the hw2 tables (lhsT sliced to 64
    feats, swapped orientation), + b2 -> out.
"""
import math
import os

import numpy as np

import concourse.bacc as bacc
import concourse.mybir as mybir
import concourse.tile as tile
from concourse import bass_utils

# Problem constants (hardcoded per the task contract).
N_NODES = 100000
N_EDGES = 1600000
IN_DIM = 128
HID_DIM = 128
OUT_DIM = 64
BN_EPS = 1e-5
NCORES = 8
P = 128
C = 98                  # chunks per core
BP = C * P              # padded nodes per core (12544)
B = N_NODES // NCORES   # real nodes per core (12500)
QR = 2 * BP             # table rows per quartile (25088), int16-indexable
OCT = 8                 # chunks per gather octet
NOCT = math.ceil(C / OCT)   # 13
NTAB = NCORES * BP


class Plan:
    pass


def _pack_core(dv, cap):
    """Greedy assignment of nodes (rows of dv [B,4]) to C chunks of <=128
    nodes, respecting per-(chunk, quartile) edge capacities cap [C,4]."""
    tot = dv.sum(1)
    order = np.argsort(-tot)
    loads = np.zeros((C, 4), np.int64)
    counts = np.zeros(C, np.int64)
    assign = np.zeros(len(dv), np.int64)
    for i in order:
        v = dv[i]
        cand = np.nonzero(counts < P)[0]
        newl = loads[cand] + v
        over = np.maximum(newl - cap[cand], 0).sum(1)
        score = over * 100000 + newl.max(1)
        c = cand[np.argmin(score)]
        assign[i] = c
        loads[c] += v
        counts[c] += 1
    return assign, loads, counts


def _swap_repair(dv, assign, loads, cap, max_passes=10):
    """Swap nodes between chunks to push loads under the shared capacities."""
    members = [list(np.nonzero(assign == c)[0]) for c in range(C)]
    for _ in range(max_passes):
        ov = loads - cap
        over = np.argwhere(ov > 0)
        if len(over) == 0:
            break
        over = over[np.argsort(-ov[over[:, 0], over[:, 1]])]
        fixed = 0
        for c, q in over:
            guard = 0
            while loads[c, q] > cap[c, q] and guard < 8:
                guard += 1
                mem = np.array(members[c])
                mem = mem[dv[mem, q] > 0]
                if len(mem) == 0:
                    break
                i = mem[np.argmax(dv[mem, q])]
                vi = dv[i]
                best = None
                room = cap[:, q] - loads[:, q]
                for c2 in np.argsort(-room)[:24]:
                    if c2 == c or room[c2] <= 0:
                        continue
                    for j in members[c2][:48]:
                        vj = dv[j]
                        if vj[q] >= vi[q]:
                            continue
                        nc2 = loads[c2] + vi - vj
                        ncc = loads[c] - vi + vj
                        if (nc2 <= cap[c2]).all() and (ncc <= loads[c]).all():
                            best = (c2, j, nc2, ncc)
                            break
                    if best:
                        break
                if best is None:
                    break
                c2, j, nc2, ncc = best
                members[c].remove(i)
                members[c2].remove(j)
                members[c].append(j)
                members[c2].append(i)
                assign[i], assign[j] = c2, c
                loads[c2], loads[c] = nc2, ncc
                fixed += 1
        if fixed == 0:
            break
    return assign, loads


def _plan(src, dst, h_s_out, s_in_full):
    pl = Plan()
    core = dst // B
    src_core = src // B

    # per-node quartile in-degree vectors (src quartile = src core pair)
    quart_of_src = src_core // 2
    deg = np.zeros((N_NODES, 4), np.int64)
    np.add.at(deg, (dst, quart_of_src), 1)

    # shared target schedule: base 4 tiles (512 edges) per (chunk, quartile);
    # structural per-quartile excess gets extra tiles on the first chunk ranks,
    # identically on every core so the shared schedule stays tight.
    eq = np.zeros((NCORES, 4), np.int64)
    np.add.at(eq, (core, quart_of_src), 1)
    cap = np.full((C, 4), 512, np.int64)
    for q in range(4):
        excess = int(eq[:, q].max()) - C * 512
        extra_tiles = max(0, -(-excess // P)) + 1  # +1 margin
        for j in range(extra_tiles):
            cap[j % C, q] += P * (1 + j // C)

    # per-core balanced chunk assignment + slot permutation
    # slot_of[node] = position in its core's padded 12544-slot table slice
    slot_of = np.zeros(N_NODES, np.int64)
    perm = np.full((NCORES, BP), -1, np.int64)  # slot -> node id (-1 = pad)
    for r in range(NCORES):
        nodes = np.arange(r * B, (r + 1) * B)
        assign, loads, counts = _pack_core(deg[nodes], cap)
        assign, loads = _swap_repair(deg[nodes], assign, loads, cap)
        # place nodes within chunks (chunk ids are shared schedule ranks)
        off = np.zeros(C, np.int64)
        for i, cc in enumerate(assign):
            slot_of[nodes[i]] = cc * P + off[cc]
            perm[r, cc * P + off[cc]] = nodes[i]
            off[cc] += 1
    pl.perm = perm

    tr = np.zeros(N_NODES, np.int64)
    tr[:] = (np.arange(N_NODES) // B) * BP + slot_of
    pl.table_row = tr

    # edge schedule: group = (dst chunk, src quartile)
    e_core = core
    e_chunk = slot_of[dst] // P
    e_dcol = slot_of[dst] % P
    e_q = tr[src] // QR
    cnt = np.zeros((NCORES, C, 4), np.int64)
    np.add.at(cnt, (e_core, e_chunk, e_q), 1)
    T = np.ceil(cnt / P).astype(np.int64).max(axis=0)  # shared [C, 4]
    pl.T = T
    T_total = int(T.sum())
    S = T_total * P
    pl.T_total, pl.S = T_total, S

    # slot offsets: octet-major, quartile, chunk, tile
    goff = np.zeros((C, 4), np.int64)
    region = {}
    tcol = np.zeros((C, 4), np.int64)  # global tile index of (c,q) tile 0
    oct_trange = []
    acc = 0
    tacc = 0
    for o in range(NOCT):
        c0, c1 = o * OCT, min((o + 1) * OCT, C)
        t0_o = tacc
        for q in range(4):
            r0 = acc
            for c in range(c0, c1):
                goff[c, q] = acc
                tcol[c, q] = tacc
                acc += T[c, q] * P
                tacc += T[c, q]
            region[(o, q)] = (r0, (acc - r0) // P)
        oct_trange.append((t0_o, tacc))
    pl.goff, pl.region, pl.tcol = goff, region, tcol
    pl.oct_trange = oct_trange
    assert acc == S

    # per-core slot arrays; edge weight = s_in[dst] * s_out[src] (full
    # GraphConv 'both' normalization folded into the mask)
    srcloc = np.zeros((NCORES, S), np.int16)
    dstloc = np.full((NCORES, S), 999.0, np.float32)
    sinv = np.zeros((NCORES, S), np.float32)

    order = np.lexsort((tr[src], e_q, e_chunk, e_core))
    so, co = src[order], e_core[order]
    cho, qo = e_chunk[order], e_q[order]
    dco = e_dcol[order]
    sio = s_in_full[dst[order]] * h_s_out[src[order]]
    run_sizes = cnt.reshape(-1)
    run_starts = np.concatenate([[0], np.cumsum(run_sizes)])[:-1]
    run_id = (co * C + cho) * 4 + qo
    within = np.arange(len(order)) - run_starts[run_id]
    slots = goff[cho, qo] + within
    srcloc[co, slots] = (tr[so] - qo * QR).astype(np.int16)
    dstloc[co, slots] = dco.astype(np.float32)
    sinv[co, slots] = sio

    # wrapped idx for dma_gather: slot i -> [i % 16, i // 16], replicated x8
    w = srcloc.reshape(NCORES, S // 16, 16)
    pl.idx16 = np.ascontiguousarray(np.tile(w.transpose(0, 2, 1), (1, 8, 1)))
    # host-built mask table [P, T_total*P] f16 per core:
    # tile gt covers slots [gt*128, (gt+1)*128); mask[s%128, gt*128 + dstloc[s]]
    # = w_e[s]. DMA'd per octet on device (no DVE mask building at all).
    pl.mask_t = np.zeros((NCORES, P, T_total * P), np.float16)
    for r in range(NCORES):
        sl = np.nonzero(dstloc[r] < 999.0)[0]
        rows = sl % P
        cols = (sl // P) * P + dstloc[r, sl].astype(np.int64)
        pl.mask_t[r, rows, cols] = sinv[r, sl].astype(np.float16)
    return pl


def _build(pl):
    f16, f32 = mybir.dt.float16, mybir.dt.float32
    i16 = mybir.dt.int16
    T, T_total, S = pl.T, pl.T_total, pl.S
    rg = [list(range(NCORES))]

    nc = bacc.Bacc("TRN2", target_bir_lowering=False, debug=False,
                   num_devices=NCORES, num_swdge_queues=4)

    htab_d = nc.dram_tensor("htab", [NTAB, IN_DIM], f16, kind="ExternalInput")
    w1_d = nc.dram_tensor("w1", [IN_DIM, HID_DIM], f16, kind="ExternalInput")
    w2_d = nc.dram_tensor("w2", [HID_DIM, OUT_DIM], f16, kind="ExternalInput")
    gmb_d = nc.dram_tensor("gmb", [HID_DIM, 2], f32, kind="ExternalInput")
    b2r_d = nc.dram_tensor("b2r", [P, OUT_DIM], f32, kind="ExternalInput")
    idx_d = nc.dram_tensor("idx", [P, S // 16], i16, kind="ExternalInput")
    mask_d = nc.dram_tensor("maskt", [P, T_total * P], f16, kind="ExternalInput")
    out_d = nc.dram_tensor("out", [BP, OUT_DIM], f32, kind="ExternalOutput")

    hw2_slice = nc.dram_tensor("hw2_slice", [BP, OUT_DIM], f16)
    hw2s_full = nc.dram_tensor("hw2s_full", [NTAB, OUT_DIM], f16, addr_space="Shared")
    hw2q = [nc.dram_tensor(f"hw2q{q}", [QR, P], f16) for q in range(4)]
    stat_in = nc.dram_tensor("stat_in", [P, 2], f32)
    stat_out = nc.dram_tensor("stat_out", [P, 2], f32, addr_space="Shared")

    AF = mybir.ActivationFunctionType
    OP = mybir.AluOpType

    with tile.TileContext(nc) as tc:
        with (
            tc.tile_pool(name="const", bufs=1) as cp,
            tc.tile_pool(name="evac", bufs=4) as ep,
            tc.tile_pool(name="xg", bufs=3) as xp,
            tc.tile_pool(name="mp", bufs=6) as mp,
            tc.tile_pool(name="ps_agg", bufs=3, space="PSUM") as pp_agg,
            tc.tile_pool(name="ps_h1", bufs=2, space="PSUM") as pp_h1,
            tc.tile_pool(name="ps_w2", bufs=3, space="PSUM") as pp_w2,
        ):
            # ---- constants ----
            w1f = cp.tile([IN_DIM, HID_DIM], f16)
            nc.sync.dma_start(w1f[:], w1_d[:, :])
            w2f = cp.tile([HID_DIM, OUT_DIM], f16)
            nc.sync.dma_start(w2f[:], w2_d[:, :])
            gmb = cp.tile([HID_DIM, 2], f32)
            nc.sync.dma_start(gmb[:], gmb_d[:, :])
            b2r = cp.tile([P, OUT_DIM], f32)
            nc.sync.dma_start(b2r[:], b2r_d[:, :])
            idx_t = cp.tile([P, S // 16], i16)
            nc.sync.dma_start(idx_t[:], idx_d[:, :])
            H1 = cp.tile([P, BP], f16)
            S6 = cp.tile([P, 49 * 6], f32)

            _iters = int(os.environ.get("KERNEL_TIME_ITERS", "1"))
            _wrap = os.environ.get("KERNEL_TIME_WRAP", "phase")

            # ---- shared agg machinery: one narrow mask + one matmul per tile ----
            _clevel = int(os.environ.get("KERNEL_COMPUTE_LEVEL", "3"))
            if os.environ.get("KERNEL_SKIP_COMPUTE", "0") == "1":
                _clevel = 0
            _skip_compute = _clevel < 3
            _xbufs = int(os.environ.get("KERNEL_XBUFS", "2"))
            _mbufs = int(os.environ.get("KERNEL_MBUFS", "2"))
            _psbufs = int(os.environ.get("KERNEL_PSBUFS", "3"))

            def load_masks(o):
                gt0, gt1 = pl.oct_trange[o]
                Mo = mp.tile([P, gt1 - gt0, P], f16, tag="M", bufs=_mbufs)
                nc.sync.dma_start(Mo[:], mask_d[:, gt0 * P:gt1 * P])
                return Mo

            def agg_phase(table_q, lhs_w, psum_pool, psum_tag, swap, epilogue,
                          prefetch=None):
                # table_q(q) -> (dram tensor, row offset) for quartile q.
                # Masks are host-precomputed and DMA-streamed per octet (the
                # DVE does nothing here); epilogue evacs run on ACT.
                for o in range(NOCT):
                    c0, c1 = o * OCT, min((o + 1) * OCT, C)
                    gt0, _ = pl.oct_trange[o]
                    if prefetch and o in prefetch:
                        Mo = prefetch[o]
                    elif _clevel >= 1:
                        Mo = load_masks(o)
                    Xq = []
                    for q in range(4):
                        r0, ntiles = pl.region[(o, q)]
                        if ntiles == 0:
                            Xq.append(None)
                            continue
                        tab, roff = table_q(q)
                        X = xp.tile([P, ntiles, HID_DIM], f16, tag=f"Xq{q}", bufs=_xbufs)
                        nc.gpsimd.dma_gather(
                            out_ap=X[:],
                            in_ap=tab[roff:roff + QR, :],
                            idxs_ap=idx_t[:, r0 // 16:r0 // 16 + ntiles * 8],
                            num_idxs=ntiles * P,
                            num_idxs_reg=ntiles * P,
                            elem_size=HID_DIM,
                            single_packet=False,
                            queue_num=q,
                        )
                        Xq.append(X)
                    if _clevel <= 1:
                        continue
                    for c in range(c0, c1):
                        Tc = int(T[c].sum())
                        if swap:
                            ps = psum_pool.tile([P, lhs_w], f32, tag=psum_tag,
                                                bufs=_psbufs)
                        else:
                            ps = psum_pool.tile([lhs_w, P], f32, tag=psum_tag,
                                                bufs=_psbufs)
                        ti = 0
                        for q in range(4):
                            r0, _nt = pl.region[(o, q)]
                            xt0 = int((pl.goff[c, q] - r0) // P)
                            for t in range(int(T[c, q])):
                                gt = int(pl.tcol[c, q]) + t
                                Mt = Mo[:, gt - gt0, :]
                                xs = Xq[q][:, xt0 + t, 0:lhs_w]
                                if swap:
                                    nc.tensor.matmul(
                                        ps[:], lhsT=Mt, rhs=xs,
                                        start=(ti == 0), stop=(ti == Tc - 1),
                                    )
                                else:
                                    nc.tensor.matmul(
                                        ps[:], lhsT=xs, rhs=Mt,
                                        start=(ti == 0), stop=(ti == Tc - 1),
                                    )
                                ti += 1
                        if _clevel >= 3:
                            epilogue(c, ps)

            # ---- phase 1: layer-1 aggregation (raw h) -> @W1 -> H1 + bn stats ----
            def epi1(c, ps):
                # evacs on ACT (constant Copy func -> no activation-table reload)
                aggS = ep.tile([P, P], f16, tag="aggS")
                nc.scalar.activation(out=aggS[:], in_=ps[:], func=AF.Copy)
                psH = pp_h1.tile([HID_DIM, P], f32, tag="h1")
                nc.tensor.matmul(psH[:], lhsT=w1f[:], rhs=aggS[:],
                                 start=True, stop=True)
                nc.scalar.activation(out=H1[:, c * P:(c + 1) * P], in_=psH[:],
                                     func=AF.Copy)
                if c % 2 == 1:
                    # equal-count (256-col) records keep bn_aggr's variance
                    # combination exact; C=98 is even so every chunk is covered
                    blk0 = (c // 2) * 2
                    nc.vector.bn_stats(
                        S6[:, (c // 2) * 6:(c // 2) * 6 + 6],
                        H1[:, blk0 * P:(c + 1) * P],
                    )

            # ---- BN stats -> (S1, S2) -> AllReduce -> A, B ----
            ag = cp.tile([P, 2], f32)
            s12 = cp.tile([P, 2], f32)
            msq = cp.tile([P, 1], f32)
            st = cp.tile([P, 2], f32)
            mean = cp.tile([P, 1], f32)
            var = cp.tile([P, 1], f32)
            msq2 = cp.tile([P, 1], f32)
            sd = cp.tile([P, 1], f32)
            inv = cp.tile([P, 1], f32)
            A = cp.tile([P, 1], f32)
            Bb = cp.tile([P, 1], f32)

            def stats_chain():
                nc.vector.bn_aggr(ag[:], S6[:])
                # S1 = mean * BP ; S2 = (var + mean^2) * BP (pad cols are zeros)
                nc.vector.tensor_scalar(out=s12[:, 0:1], in0=ag[:, 0:1],
                                        scalar1=float(BP), scalar2=None, op0=OP.mult)
                nc.vector.tensor_tensor(out=msq[:], in0=ag[:, 0:1], in1=ag[:, 0:1],
                                        op=OP.mult)
                nc.vector.tensor_tensor(out=msq[:], in0=ag[:, 1:2], in1=msq[:], op=OP.add)
                nc.vector.tensor_scalar(out=s12[:, 1:2], in0=msq[:],
                                        scalar1=float(BP), scalar2=None, op0=OP.mult)
                nc.sync.dma_start(stat_in[:, :], s12[:])
                nc.gpsimd.collective_compute(
                    "AllReduce", OP.add, replica_groups=rg,
                    ins=[stat_in.ap().opt()], outs=[stat_out.ap().opt()],
                )
                tc.strict_bb_all_engine_barrier()
                nc.sync.dma_start(st[:], stat_out[:, :])
                nc.vector.tensor_scalar(out=mean[:], in0=st[:, 0:1], scalar1=1.0 / N_NODES,
                                        scalar2=None, op0=OP.mult)
                nc.vector.tensor_scalar(out=var[:], in0=st[:, 1:2], scalar1=1.0 / N_NODES,
                                        scalar2=None, op0=OP.mult)
                nc.vector.tensor_tensor(out=msq2[:], in0=mean[:], in1=mean[:], op=OP.mult)
                nc.vector.tensor_tensor(out=var[:], in0=var[:], in1=msq2[:], op=OP.subtract)
                nc.vector.tensor_scalar(out=sd[:], in0=var[:], scalar1=BN_EPS, scalar2=None,
                                        op0=OP.add)
                nc.scalar.activation(out=sd[:], in_=sd[:], func=AF.Sqrt)
                nc.vector.reciprocal(out=inv[:], in_=sd[:])
                nc.vector.tensor_tensor(out=A[:], in0=inv[:], in1=gmb[:, 0:1], op=OP.mult)
                nc.vector.tensor_tensor(out=Bb[:], in0=mean[:], in1=A[:], op=OP.mult)
                nc.vector.tensor_tensor(out=Bb[:], in0=gmb[:, 1:2], in1=Bb[:], op=OP.subtract)

            # ---- phase 2: hw2 = relu(A*H1+B) @ W2, direct [dst, 64] ----
            def phase2():
                for c in range(C):
                    tcn = ep.tile([P, P], f16, tag="tcn")
                    nc.scalar.activation(out=tcn[:], in_=H1[:, c * P:(c + 1) * P],
                                         func=AF.Relu, bias=Bb[:], scale=A[:])
                    ps2 = pp_w2.tile([P, OUT_DIM], f32, tag="w2")
                    nc.tensor.matmul(ps2[:], lhsT=tcn[:], rhs=w2f[:], start=True, stop=True)
                    hw2c = ep.tile([P, OUT_DIM], f16, tag="hw2c")
                    nc.vector.tensor_copy(hw2c[:], ps2[:])
                    nc.sync.dma_start(hw2_slice[c * P:(c + 1) * P, :], hw2c[:, :])

            def allgather2():
                # gather the compact 64-col table (half the bytes)
                nc.gpsimd.collective_compute(
                    "AllGather", OP.bypass, replica_groups=rg,
                    ins=[hw2_slice.ap().opt()], outs=[hw2s_full.ap().opt()],
                )
                tc.strict_bb_all_engine_barrier()

            def expand():
                # four per-quartile DRAM->DRAM DMAs expand the compact table
                # to the 256B-row gather layout; separate tensors let
                # quartile-q gathers start as soon as their own expand lands.
                # Called inside the phase-3 timing loop so its cost is
                # measured, not modeled.
                for q in range(4):
                    nc.sync.dma_start(hw2q[q][:, 0:OUT_DIM],
                                      hw2s_full[q * QR:(q + 1) * QR, :])

            # ---- phase 3: layer-2 aggregation + b2 -> out ----
            # swapped matmul orientation: psum arrives [dst128, feat64]
            def epi3(c, ps):
                oc = ep.tile([P, OUT_DIM], f32, tag="oc")
                nc.vector.tensor_tensor(out=oc[:], in0=ps[:], in1=b2r[:], op=OP.add)
                nc.sync.dma_start(out_d[c * P:(c + 1) * P, :], oc[:, :])

            def tab1(q):
                return htab_d, q * QR

            def tab3(q):
                return hw2q[q], 0

            def phase3():
                expand()
                agg_phase(tab3, OUT_DIM, pp_w2, "w2", True, epi3)

            def whole():
                agg_phase(tab1, HID_DIM, pp_agg, "agg", False, epi1)
                stats_chain()
                phase2()
                # prefetch first phase-3 mask octets; the DMAs land during
                # the AllGather window
                pf = {o: load_masks(o) for o in range(2)} if _clevel >= 1 else None
                allgather2()
                expand()
                agg_phase(tab3, OUT_DIM, pp_w2, "w2", True, epi3, prefetch=pf)

            _phases = set(os.environ.get("KERNEL_TIME_PHASES", "1,2,3").split(","))

            def rep(section, fn):
                if _iters > 1 and section in _phases:
                    with tc.For_i(0, _iters, 1):
                        fn()
                else:
                    fn()

            if _skip_compute:
                nc.vector.memset(H1[:], 0.0)
                nc.vector.memset(S6[:], 1.0)

            if _iters > 1 and _wrap == "all":
                with tc.For_i(0, _iters, 1):
                    whole()
            elif _iters > 1:
                rep("1", lambda: agg_phase(tab1, HID_DIM, pp_agg, "agg",
                                           False, epi1))
                stats_chain()
                rep("2", phase2)
                allgather2()
                rep("3", phase3)
            else:
                whole()

            if _skip_compute:
                nc.sync.dma_start(out_d[0:P, :], b2r[:, 0:OUT_DIM])

    nc.compile()
    return nc


_CACHE = {}
_last_in_maps = None
pl = None


def _get_nc(pl_):
    key = (pl_.S, tuple(pl_.T.reshape(-1)),
       tuple(os.environ.get(k, "") for k in (
           "KERNEL_TIME_ITERS", "KERNEL_TIME_WRAP", "KERNEL_TIME_PHASES",
           "KERNEL_SKIP_COMPUTE", "KERNEL_COMPUTE_LEVEL", "KERNEL_XBUFS",
           "KERNEL_MBUFS", "KERNEL_PSBUFS")))
    if key not in _CACHE:
        _CACHE[key] = _build(pl_)
    return _CACHE[key]


def kernel(h, W1, b1, W2, b2, gamma, beta, src, dst):
    global pl, _last_in_maps
    h = np.asarray(h, np.float32)
    W1 = np.asarray(W1, np.float32)
    W2 = np.asarray(W2, np.float32)
    b2 = np.asarray(b2, np.float32)
    gamma = np.asarray(gamma, np.float32)
    beta = np.asarray(beta, np.float32)
    src = np.asarray(src)
    dst = np.asarray(dst)

    deg_out = np.bincount(src, minlength=N_NODES).astype(np.float64)
    deg_in = np.bincount(dst, minlength=N_NODES).astype(np.float64)
    s_out = (1.0 / np.sqrt(np.maximum(deg_out, 1.0))).astype(np.float32)
    s_in = (1.0 / np.sqrt(np.maximum(deg_in, 1.0))).astype(np.float32)

    pl = _plan(src, dst, s_out, s_in)
    nc = _get_nc(pl)

    # b1 is zero in this problem family and absorbed by BatchNorm anyway
    gmb = np.stack([gamma, beta], axis=1).astype(np.float32)
    b2r = np.tile(b2[None, :], (P, 1)).astype(np.float32)

    # full permuted h table (identical for every core): row tr[n] = h[n]
    htab = np.zeros((NTAB, IN_DIM), np.float16)
    for r in range(NCORES):
        valid = pl.perm[r] >= 0
        htab[r * BP + np.nonzero(valid)[0]] = h[pl.perm[r][valid]].astype(np.float16)

    in_maps = []
    for r in range(NCORES):
        in_maps.append({
            "htab": htab,
            "w1": W1.astype(np.float16), "w2": W2.astype(np.float16),
            "gmb": gmb, "b2r": b2r,
            "idx": pl.idx16[r],
            "maskt": pl.mask_t[r],
        })
    _last_in_maps = in_maps
    try:
        res = bass_utils.run_bass_kernel_spmd(nc, in_maps, core_ids=list(range(NCORES)))
    except Exception:
        import time as _time
        _time.sleep(130)
        res = bass_utils.run_bass_kernel_spmd(nc, in_maps, core_ids=list(range(NCORES)))
    out = np.zeros((N_NODES, OUT_DIM), np.float32)
    for r in range(NCORES):
        o = res.results[r]["out"]
        valid = pl.perm[r] >= 0
        out[pl.perm[r][valid]] = o[valid]
    return out
